# revision 1
# baseline (speedup 1.0000x reference)
"""Sparse (sliding-window + sink) GQA attention on 8 NeuronCores.

Sharding: tensor-parallel over heads. Core c owns q-heads {2c, 2c+1} and
kv-head c//2. Each core computes its heads' attention and a partial
output projection (wo columns for its heads); host sums the 8 partials.

All big matmuls run as float32r (full-rate fp32 PE mode, N>=256).
Attention is computed in transposed orientation ST[k, q] so the P@V
contraction needs no on-chip transposes of the probability matrix; the
softmax denominator comes from a ones-vector matmul, and the final
normalization is folded into the PSUM->SBUF eviction of the output.
The kernel returns out^T; the host transposes back.
"""

import numpy as np
from contextlib import ExitStack

import concourse.bass as bass
import concourse.bacc as bacc
import concourse.mybir as mybir
import concourse.tile as tile
from concourse.bass_utils import run_bass_kernel_spmd

S = 2048
H = 16
KVH = 4
D = 128
HID = H * D
WIN = 1024
EPS = 1e-5
NCORES = 8
F32 = mybir.dt.float32
F32R = mybir.dt.float32r
AF = mybir.ActivationFunctionType
SCALE = 1.0 / float(np.sqrt(D))
NEG = -1e9
NSC = S // 128  # 16 s-chunks
NEC = HID // 128  # 16 e-chunks


def _r(ap):
    return ap


def _build_kernel():
    nc = bacc.Bacc("TRN2", target_bir_lowering=False, debug=False)

    xT = nc.dram_tensor("xT", [HID, S], F32R, kind="ExternalInput").ap()
    wqT = nc.dram_tensor("wqT", [HID, 256], F32R, kind="ExternalInput").ap()
    wkvT = nc.dram_tensor("wkvT", [HID, 256], F32R, kind="ExternalInput").ap()
    woT = nc.dram_tensor("woT", [256, HID], F32R, kind="ExternalInput").ap()
    cos3 = nc.dram_tensor("cos3", [S, 256], F32, kind="ExternalInput").ap()
    sin3 = nc.dram_tensor("sin3", [S, 256], F32, kind="ExternalInput").ap()
    sinks2 = nc.dram_tensor("sinks2", [1, 2], F32, kind="ExternalInput").ap()
    maskd = nc.dram_tensor("maskd", [128, 128], F32, kind="ExternalInput").ap()
    maske = nc.dram_tensor("maske", [128, 128], F32, kind="ExternalInput").ap()
    ones1 = nc.dram_tensor("ones1", [128, 1], F32R, kind="ExternalInput").ap()
    identf = nc.dram_tensor("identf", [128, 128], F32, kind="ExternalInput").ap()
    outT = nc.dram_tensor("outT", [HID, S], F32, kind="ExternalOutput").ap()

    with tile.TileContext(nc) as tc:
        with ExitStack() as ctx:
            _emit(ctx, tc, nc, xT, wqT, wkvT, woT, cos3, sin3, sinks2,
                  maskd, maske, ones1, identf, outT)
    nc.compile()
    return nc


def _emit(ctx, tc, nc, xT, wqT, wkvT, woT, cos3, sin3, sinks2, maskd, maske,
          ones1, identf, outT):
    # persistent tensors
    pers = ctx.enter_context(tc.tile_pool(name="pers", bufs=1))
    wpool = ctx.enter_context(tc.tile_pool(name="wq", bufs=NEC))
    wkpool = ctx.enter_context(tc.tile_pool(name="wkv", bufs=NEC))
    # streaming pools
    xtpool = ctx.enter_context(tc.tile_pool(name="xtb", bufs=24))
    qkpool = ctx.enter_context(tc.tile_pool(name="qk", bufs=3))
    ctpool = ctx.enter_context(tc.tile_pool(name="ct", bufs=3))
    smpool = ctx.enter_context(tc.tile_pool(name="small", bufs=6))
    espool = ctx.enter_context(tc.tile_pool(name="es", bufs=8))
    dnpool = ctx.enter_context(tc.tile_pool(name="dn", bufs=2))
    dbpool = ctx.enter_context(tc.tile_pool(name="db", bufs=2))
    otpool = ctx.enter_context(tc.tile_pool(name="ot", bufs=4))
    # psum pools
    psA = ctx.enter_context(tc.tile_pool(name="psA", bufs=3, space="PSUM"))
    psB = ctx.enter_context(tc.tile_pool(name="psB", bufs=3, space="PSUM"))
    psC = ctx.enter_context(tc.tile_pool(name="psC", bufs=2, space="PSUM"))

    QT = [pers.tile([128, S], F32R, tag=f"QT{h}", name=f"QT{h}") for h in range(2)]
    KT = pers.tile([128, S], F32R, tag="KT")
    Vb = pers.tile([128, S], F32R, tag="Vb")
    attnT = [pers.tile([128, S], F32R, tag=f"attnT{h}", name=f"attnT{h}") for h in range(2)]
    woTs = [pers.tile([128, S], F32R, tag=f"woT{i}", name=f"woT{i}") for i in range(2)]
    md = pers.tile([128, 128], F32, tag="maskd")
    me = pers.tile([128, 128], F32, tag="maske")
    idtf = pers.tile([128, 128], F32, tag="identf")
    ones = pers.tile([128, 1], F32R, tag="ones")
    sks = pers.tile([1, 2], F32, tag="sinks")
    epsb = pers.tile([128, 1], F32, tag="epsb")
    esink = pers.tile([1, 2], F32, tag="esink")

    nc.sync.dma_start(idtf[:], identf[:])
    nc.vector.memset(epsb[:], EPS)
    wqb = pers.tile([128, NEC * 256], F32R, tag="wqb")
    wkvb = pers.tile([128, NEC * 256], F32R, tag="wkvb")

    def emit_weight_loads():
        nc.sync.dma_start(md[:], maskd[:])
        nc.sync.dma_start(me[:], maske[:])
        nc.sync.dma_start(sks[:], sinks2[:])
        nc.sync.dma_start(ones[:], ones1[:])

        for i in range(2):
            for q4 in range(2):
                nc.sync.dma_start(woTs[i][:, q4 * 1024:(q4 + 1) * 1024],
                                  woT[i * 128:(i + 1) * 128,
                                      q4 * 1024:(q4 + 1) * 1024])

    # ---- Phase 2: load xT row-blocks, project, norm, rope ----
    def emit_xload(sb, ecs):
        tiles = []
        for ec in ecs:
            xt = xtpool.tile([128, 512], F32R, tag="xtb",
                             name=f"xt{sb}_{ec}")
            nc.sync.dma_start(xt[:], xT[ec * 128:(ec + 1) * 128,
                                        sb * 512:(sb + 1) * 512])
            tiles.append(xt)
        return tiles

    def emit_proj(sc, tiles, sq):
        qp = psB.tile([128, 256], F32, tag="psB", name=f"qp{sc}")
        kvp = psB.tile([128, 256], F32, tag="psB", name=f"kvp{sc}")
        a = sq * 128
        for ec in range(NEC):
            nc.tensor.matmul(qp[:], tiles[ec][:, a:a + 128],
                             _r(wqb[:, ec * 256:(ec + 1) * 256]),
                             start=(ec == 0), stop=(ec == NEC - 1))
        for ec in range(NEC):
            nc.tensor.matmul(kvp[:], tiles[ec][:, a:a + 128],
                             _r(wkvb[:, ec * 256:(ec + 1) * 256]),
                             start=(ec == 0), stop=(ec == NEC - 1))
        qk = qkpool.tile([128, 384], F32, tag="qk", name=f"qk{sc}")
        if sc % 2 == 0:
            nc.scalar.copy(qk[:, 0:256], qp[:])
            nc.vector.tensor_copy(qk[:, 256:384], kvp[:, 0:128])
            nc.vector.tensor_copy(Vb[:, sc * 128:(sc + 1) * 128],
                                  kvp[:, 128:256])
        else:
            nc.vector.tensor_copy(qk[:, 0:256], qp[:])
            nc.scalar.copy(qk[:, 256:384], kvp[:, 0:128])
            nc.scalar.copy(Vb[:, sc * 128:(sc + 1) * 128], kvp[:, 128:256])
        # rms norm (q0, q1, k)
        for hh in range(3):
            o = hh * 128
            sq_t = smpool.tile([128, 128], F32, tag="sq", name=f"sqr{sc}_{hh}")
            ss = smpool.tile([128, 1], F32, tag="ss", name=f"ss{sc}_{hh}")
            nc.scalar.activation(sq_t[:], qk[:, o:o + 128], AF.Square,
                                 accum_out=ss[:])
            rs = smpool.tile([128, 1], F32, tag="rs", name=f"rs{sc}_{hh}")
            nc.scalar.activation(rs[:], ss[:], AF.Sqrt, bias=epsb[:],
                                 scale=1.0 / D)
            iv = smpool.tile([128, 1], F32, tag="iv", name=f"iv{sc}_{hh}")
            nc.vector.reciprocal(iv[:], rs[:])
            nc.vector.tensor_scalar_mul(qk[:, o:o + 128], qk[:, o:o + 128],
                                        iv[:])
        # rope
        rot = qkpool.tile([128, 384], F32, tag="rot", name=f"rot{sc}")
        for hh in range(3):
            o = hh * 128
            nc.vector.tensor_scalar_mul(rot[:, o:o + 64],
                                        qk[:, o + 64:o + 128], -1.0)
            nc.vector.tensor_copy(rot[:, o + 64:o + 128], qk[:, o:o + 64])
        cq = ctpool.tile([128, 384], F32, tag="cq", name=f"cq{sc}")
        nc.sync.dma_start(cq[:, 0:128], cos3[sc * 128:(sc + 1) * 128, 0:128])
        nc.sync.dma_start(cq[:, 256:384],
                          cos3[sc * 128:(sc + 1) * 128, 128:256])
        nc.gpsimd.tensor_copy(cq[:, 128:256], cq[:, 0:128])
        sq_ = ctpool.tile([128, 384], F32, tag="sq_", name=f"sqt{sc}")
        nc.sync.dma_start(sq_[:, 0:128], sin3[sc * 128:(sc + 1) * 128, 0:128])
        nc.sync.dma_start(sq_[:, 256:384],
                          sin3[sc * 128:(sc + 1) * 128, 128:256])
        nc.gpsimd.tensor_copy(sq_[:, 128:256], sq_[:, 0:128])
        qkr = qkpool.tile([128, 384], F32, tag="qkr", name=f"qkr{sc}")
        nc.vector.tensor_mul(qkr[:], qk[:], cq[:])
        nc.vector.tensor_mul(rot[:], rot[:], sq_[:])
        nc.vector.tensor_add(qkr[:], qkr[:], rot[:])
        return qkr

    def emit_qtrans(sc, qkr):
        dests = [QT[0], QT[1], KT]
        for hh in range(3):
            pt = psC.tile([128, 128], F32, tag="psC", name=f"ptq{sc}_{hh}")
            nc.tensor.transpose(pt[:], qkr[:, hh * 128:(hh + 1) * 128],
                                idtf[:])
            if hh % 2 == 0:
                nc.scalar.copy(dests[hh][:, sc * 128:(sc + 1) * 128], pt[:])
            else:
                nc.vector.tensor_copy(dests[hh][:, sc * 128:(sc + 1) * 128],
                                      pt[:])

    wq3 = wqT.rearrange("(n p) d -> p n d", p=128)
    wk3 = wkvT.rearrange("(n p) d -> p n d", p=128)
    qb3 = wqb[:].rearrange("p (n d) -> p n d", n=NEC)
    kb3 = wkvb[:].rearrange("p (n d) -> p n d", n=NEC)
    tiles_cur = []
    for q4 in range(4):
        tiles_cur += emit_xload(0, range(4 * q4, 4 * q4 + 4))
        nc.sync.dma_start(qb3[:, q4 * 4:(q4 + 1) * 4, :],
                          wq3[:, q4 * 4:(q4 + 1) * 4, :])
        nc.sync.dma_start(kb3[:, q4 * 4:(q4 + 1) * 4, :],
                          wk3[:, q4 * 4:(q4 + 1) * 4, :])
    emit_weight_loads()
    qkr_prev = None
    for sb in range(4):
        nxt = []
        for sq in range(4):
            sc = sb * 4 + sq
            if sb + 1 < 4:
                nxt += emit_xload(sb + 1, range(4 * sq, 4 * sq + 4))
            qkr_cur = emit_proj(sc, tiles_cur, sq)
            if qkr_prev is not None:
                emit_qtrans(sc - 1, qkr_prev)
            qkr_prev = qkr_cur
        tiles_cur = nxt
    emit_qtrans(NSC - 1, qkr_prev)

    # ---- Phase 3: attention, transposed orientation ----
    nc.scalar.activation(esink[:], sks[:], AF.Exp)
    def emit_scores(h, t, kc):
        jlo = max(0, kc - 4 * t)
        jhi = min(3, kc + 8 - 4 * t)
        a, b = jlo * 128, (jhi + 1) * 128
        qa, qb = t * 512 + a, t * 512 + b
        sp = psA.tile([128, 512], F32, tag="psA", name=f"sp{h}_{t}_{kc}")
        nc.tensor.matmul(sp[:, a:b], _r(KT[:, kc * 128:(kc + 1) * 128]),
                         _r(QT[h][:, qa:qb]), start=True, stop=True)
        return sp, a, b

    def emit_wo(t):
        for jc in range(16):
            po = psC.tile([128, 512], F32, tag="psC", name=f"po{jc}_{t}")
            for ic in range(2):
                nc.tensor.matmul(
                    po[:], _r(woTs[ic][:, jc * 128:(jc + 1) * 128]),
                    _r(attnT[ic][:, t * 512:(t + 1) * 512]),
                    start=(ic == 0), stop=(ic == 1))
            ot = otpool.tile([128, 512], F32, tag="ot", name=f"ot{jc}_{t}")
            if (jc + t) % 2 == 0:
                nc.scalar.copy(ot[:], po[:])
            else:
                nc.vector.tensor_copy(ot[:], po[:])
            nc.sync.dma_start(outT[jc * 128:(jc + 1) * 128,
                                   t * 512:(t + 1) * 512], ot[:])

    groups = []
    for t in range(4):
        for h in range(2):
            groups.append((h, t, list(range(max(0, 4 * t - 8), 4 * (t + 1)))))

    all_work = []  # flat list of (h, t, kc)
    for h, t, kcs in groups:
        for kc in kcs:
            all_work.append((h, t, kc))

    pending = {}  # (h,t,kc) -> (sp, a, b)
    LOOKAHEAD = 5
    wi = 0  # next work item to prefetch

    def prefetch(upto):
        nonlocal wi
        while wi < len(all_work) and wi < upto:
            hh, tt, kk = all_work[wi]
            pending[(hh, tt, kk)] = emit_scores(hh, tt, kk)
            wi += 1

    idx = 0
    for gi, (h, t, kcs) in enumerate(groups):
        op = psB.tile([128, 512], F32, tag="psB", name=f"op{h}_{t}")
        dp = psB.tile([1, 512], F32, tag="psB", name=f"dp{h}_{t}")
        for i, kc in enumerate(kcs):
            prefetch(idx + 1 + LOOKAHEAD)
            sp, a, b = pending.pop((h, t, kc))
            idx += 1
            j = kc - 4 * t
            if 0 <= j < 4:
                nc.vector.tensor_add(sp[:, j * 128:(j + 1) * 128],
                                     sp[:, j * 128:(j + 1) * 128], md[:])
            j2 = kc + 8 - 4 * t
            if 0 <= j2 < 4:
                nc.vector.tensor_add(sp[:, j2 * 128:(j2 + 1) * 128],
                                     sp[:, j2 * 128:(j2 + 1) * 128],
                                     me[:])
            es = espool.tile([128, 512], F32R, tag="es",
                             name=f"es{h}_{t}_{kc}")
            nc.scalar.activation(es[:, a:b], sp[:, a:b], AF.Exp,
                                 scale=SCALE)
            first, last = (i == 0), (i == len(kcs) - 1)
            nc.tensor.matmul(dp[:, a:b], _r(ones[:]), _r(es[:, a:b]),
                             start=first, stop=last)
            nc.tensor.matmul(op[:, a:b],
                             _r(Vb[:, kc * 128:(kc + 1) * 128]),
                             _r(es[:, a:b]), start=first, stop=last)
        dn = dnpool.tile([1, 512], F32, tag="dn", name=f"dn{h}_{t}")
        nc.vector.tensor_scalar_add(dn[:], dp[:], esink[:, h:h + 1])
        nc.vector.reciprocal(dn[:], dn[:])
        db = dbpool.tile([128, 512], F32, tag="db", name=f"db{h}_{t}")
        nc.gpsimd.partition_broadcast(db[:], dn[:])
        nc.vector.tensor_mul(attnT[h][:, t * 512:(t + 1) * 512], op[:],
                             db[:])
        if gi >= 2 and gi % 2 == 0:
            emit_wo(t - 1)
        if gi == len(groups) - 1:
            emit_wo(3)


_NC_CACHE = {}


def _get_nc():
    if "nc" not in _NC_CACHE:
        _NC_CACHE["nc"] = _build_kernel()
    return _NC_CACHE["nc"]


def kernel(x, cos, sin, wq, wk, wv, wo, sinks, q_norm_w, k_norm_w):
    x = np.asarray(x, np.float32).reshape(S, HID)
    xTh = np.ascontiguousarray(x.T)
    cos = np.asarray(cos, np.float32)
    sin = np.asarray(sin, np.float32)
    wq = np.asarray(wq, np.float32)
    wk = np.asarray(wk, np.float32)
    wv = np.asarray(wv, np.float32)
    wo = np.asarray(wo, np.float32)
    sinks = np.asarray(sinks, np.float32)
    qw = np.asarray(q_norm_w, np.float32)
    kw = np.asarray(k_norm_w, np.float32)

    qwr = np.roll(qw, -64)
    kwr = np.roll(kw, -64)
    cos3 = np.ascontiguousarray(
        np.concatenate([cos * qw, cos * kw], axis=1))
    sin3 = np.ascontiguousarray(
        np.concatenate([sin * qwr, sin * kwr], axis=1))
    kk = np.arange(128)[:, None]
    qq = np.arange(128)[None, :]
    maskd = np.where(kk <= qq, 0.0, NEG).astype(np.float32)
    maske = np.where(kk >= qq, 0.0, NEG).astype(np.float32)
    ident = np.eye(128, dtype=np.float32)

    in_maps = []
    for c in range(NCORES):
        kvh = c // 2
        wqT = np.ascontiguousarray(wq[2 * c * 128:(2 * c + 2) * 128, :].T)
        wkv = np.concatenate([wk[kvh * 128:(kvh + 1) * 128, :],
                              wv[kvh * 128:(kvh + 1) * 128, :]], axis=0)
        wkvT = np.ascontiguousarray(wkv.T)
        woT = np.ascontiguousarray(wo[:, c * 256:(c + 1) * 256].T)
        in_maps.append(dict(
            xT=xTh, wqT=wqT, wkvT=wkvT, woT=woT, cos3=cos3, sin3=sin3,
            sinks2=np.ascontiguousarray(sinks[2 * c:2 * c + 2].reshape(1, 2)),
            maskd=maskd, maske=maske, identf=ident,
            ones1=np.ones((128, 1), np.float32)))

    nc = _get_nc()
    res = run_bass_kernel_spmd(nc, in_maps, core_ids=list(range(NCORES)))
    total = res.results[0]["outT"]
    for c in range(1, NCORES):
        total = total + res.results[c]["outT"]
    return np.ascontiguousarray(total.T).reshape(1, S, HID)



# revision 53
# speedup vs baseline: 1.1920x; 1.1920x over previous
"""Sparse (sliding-window + sink) GQA attention on 8 NeuronCores.

Sharding: tensor-parallel over heads. Core c owns q-heads {2c, 2c+1} and
kv-head c//2. Each core computes its heads' attention and a partial
output projection (wo columns for its heads); host sums the 8 partials.

Data plane is bf16 (matmul inputs, DMA traffic); softmax and PSUM stay
f32. Attention runs in transposed orientation ST[k, q] so the P@V
contraction needs no on-chip transposes of the probability matrix; the
softmax denominator comes from a ones-vector matmul, and the final
normalization is folded into the PSUM->SBUF eviction of the output.
The kernel returns out^T in bf16; the host upcasts, sums and
transposes back.
"""

import numpy as np
from contextlib import ExitStack

import ml_dtypes
import concourse.bass as bass
import concourse.bacc as bacc
import concourse.mybir as mybir
import concourse.tile as tile
from concourse.bass_utils import run_bass_kernel_spmd

S = 2048
H = 16
KVH = 4
D = 128
HID = H * D
WIN = 1024
EPS = 1e-5
NCORES = 8
F32 = mybir.dt.float32
BF16 = mybir.dt.bfloat16
AF = mybir.ActivationFunctionType
SCALE = 1.0 / float(np.sqrt(D))
NEG = -1e9
NSC = S // 128  # 16 s-chunks
NEC = HID // 128  # 16 e-chunks
NBF = ml_dtypes.bfloat16


def _build_kernel():
    nc = bacc.Bacc("TRN2", target_bir_lowering=False, debug=False)

    xT = nc.dram_tensor("xT", [HID, S], BF16, kind="ExternalInput").ap()
    wqkvT = nc.dram_tensor("wqkvT", [HID, 512], BF16, kind="ExternalInput").ap()
    woT = nc.dram_tensor("woT", [256, HID], BF16, kind="ExternalInput").ap()
    cs3 = nc.dram_tensor("cs3", [S, 384], BF16, kind="ExternalInput").ap()
    sn3 = nc.dram_tensor("sn3", [S, 384], BF16, kind="ExternalInput").ap()
    sinks2 = nc.dram_tensor("sinks2", [1, 2], F32, kind="ExternalInput").ap()
    maskd = nc.dram_tensor("maskd", [128, 128], F32, kind="ExternalInput").ap()
    maske = nc.dram_tensor("maske", [128, 128], F32, kind="ExternalInput").ap()
    ones1 = nc.dram_tensor("ones1", [128, 1], BF16, kind="ExternalInput").ap()
    identf = nc.dram_tensor("identf", [128, 128], F32, kind="ExternalInput").ap()
    outT = nc.dram_tensor("outT", [HID, S], BF16, kind="ExternalOutput").ap()

    with tile.TileContext(nc) as tc:
        with ExitStack() as ctx:
            _emit(ctx, tc, nc, xT, wqkvT, woT, cs3, sn3, sinks2,
                  maskd, maske, ones1, identf, outT)
    nc.compile()
    return nc


def _emit(ctx, tc, nc, xT, wqkvT, woT, cs3, sn3, sinks2, maskd, maske,
          ones1, identf, outT):
    # persistent tensors
    pers = ctx.enter_context(tc.tile_pool(name="pers", bufs=1))
    # streaming pools
    qkpool = ctx.enter_context(tc.tile_pool(name="qk", bufs=4))
    smpool = ctx.enter_context(tc.tile_pool(name="small", bufs=6))
    espool = ctx.enter_context(tc.tile_pool(name="es", bufs=10))
    dnpool = ctx.enter_context(tc.tile_pool(name="dn", bufs=2))
    dbpool = ctx.enter_context(tc.tile_pool(name="db", bufs=2))
    otpool = ctx.enter_context(tc.tile_pool(name="ot", bufs=4))
    # psum pools
    psA = ctx.enter_context(tc.tile_pool(name="psA", bufs=3, space="PSUM"))
    psB = ctx.enter_context(tc.tile_pool(name="psB", bufs=3, space="PSUM"))
    psC = ctx.enter_context(tc.tile_pool(name="psC", bufs=2, space="PSUM"))

    QT = [pers.tile([128, S], BF16, tag=f"QT{h}", name=f"QT{h}") for h in range(2)]
    KT = pers.tile([128, S], BF16, tag="KT")
    Vb = pers.tile([128, S], BF16, tag="Vb")
    attnT = [pers.tile([128, S], BF16, tag=f"attnT{h}", name=f"attnT{h}") for h in range(2)]
    woTs = [pers.tile([128, S], BF16, tag=f"woT{i}", name=f"woT{i}") for i in range(2)]
    md = pers.tile([128, 128], F32, tag="maskd")
    me = pers.tile([128, 128], F32, tag="maske")
    idtf = pers.tile([128, 128], F32, tag="identf")
    ones = pers.tile([128, 1], BF16, tag="ones")
    sks = pers.tile([1, 2], F32, tag="sinks")
    epsb = pers.tile([128, 1], F32, tag="epsb")
    esink = pers.tile([1, 2], F32, tag="esink")

    # x blocks: 8 blocks of 256 seq, each [128 hid-part, 16 ec, 256 seq]
    xb = [pers.tile([128, NEC * 256], BF16, tag=f"xb{b}", name=f"xb{b}")
          for b in range(8)]
    xb3 = [t[:].rearrange("p (n d) -> p n d", n=NEC) for t in xb]
    wb = pers.tile([128, NEC * 512], BF16, tag="wb")
    wb3 = wb[:].rearrange("p (n d) -> p n d", n=NEC)
    csb = pers.tile([128, NSC * 384], BF16, tag="csb")
    csb3 = csb[:].rearrange("p (n d) -> p n d", n=NSC)
    snb = pers.tile([128, NSC * 384], BF16, tag="snb")
    snb3 = snb[:].rearrange("p (n d) -> p n d", n=NSC)

    nc.vector.memset(epsb[:], EPS)

    # ---- Phase 1: DMA loads (weights first: they gate the first chain) ----
    xT3 = xT.rearrange("(n p) s -> p n s", p=128)
    wq3 = wqkvT.rearrange("(n p) d -> p n d", p=128)
    cs3r = cs3.rearrange("(n p) d -> p n d", p=128)
    sn3r = sn3.rearrange("(n p) d -> p n d", p=128)
    nc.sync.dma_start(wb3[:, 0:1, :], wq3[:, 0:1, :])
    nc.sync.dma_start(xb3[0][:, 0:1, :], xT3[:, 0:1, 0:256])
    nc.sync.dma_start(wb3[:, 1:4, :], wq3[:, 1:4, :])
    nc.sync.dma_start(xb3[0][:, 1:8, :], xT3[:, 1:8, 0:256])
    nc.sync.dma_start(wb3[:, 4:8, :], wq3[:, 4:8, :])
    nc.sync.dma_start(xb3[0][:, 8:16, :], xT3[:, 8:16, 0:256])
    nc.sync.dma_start(wb3[:, 8:12, :], wq3[:, 8:12, :])
    nc.sync.dma_start(wb3[:, 12:16, :], wq3[:, 12:16, :])
    nc.sync.dma_start(idtf[:], identf[:])
    nc.sync.dma_start(csb3[:, 0:4, :], cs3r[:, 0:4, :])
    nc.sync.dma_start(snb3[:, 0:4, :], sn3r[:, 0:4, :])
    for b in range(1, 8):
        nc.sync.dma_start(xb3[b][:], xT3[:, :, b * 256:(b + 1) * 256])
        if b == 2:
            nc.sync.dma_start(csb3[:, 4:8, :], cs3r[:, 4:8, :])
            nc.sync.dma_start(snb3[:, 4:8, :], sn3r[:, 4:8, :])
            nc.sync.dma_start(md[:], maskd[:])
            nc.sync.dma_start(me[:], maske[:])
            nc.sync.dma_start(sks[:], sinks2[:])
            nc.sync.dma_start(ones[:], ones1[:])
        if b == 4:
            nc.sync.dma_start(csb3[:, 8:16, :], cs3r[:, 8:16, :])
            nc.sync.dma_start(snb3[:, 8:16, :], sn3r[:, 8:16, :])
    for i in range(2):
        nc.sync.dma_start(woTs[i][:], woT[i * 128:(i + 1) * 128, :])

    # ---- Phase 2: project, norm, rope, transpose ----
    def emit_proj(sc):
        b, lo = sc // 2, (sc % 2) * 128
        qkvp = psB.tile([128, 512], F32, tag="psB", name=f"qkvp{sc}")
        for ec in range(NEC):
            nc.tensor.matmul(qkvp[:], xb3[b][:, ec, lo:lo + 128],
                             wb3[:, ec, :],
                             start=(ec == 0), stop=(ec == NEC - 1))
        # PSUM evictions: ACT/DVE only (GPSIMD cannot access PSUM)
        qk = qkpool.tile([128, 384], BF16, tag="qk", name=f"qk{sc}")
        nc.scalar.copy(qk[:, 0:256], qkvp[:, 0:256])
        nc.vector.tensor_copy(qk[:, 256:384], qkvp[:, 256:384])
        nc.scalar.copy(Vb[:, sc * 128:(sc + 1) * 128], qkvp[:, 384:512])
        # rms norm (q0, q1, k)
        for hh in range(3):
            o = hh * 128
            sq_t = smpool.tile([128, 128], F32, tag="sq", name=f"sqr{sc}_{hh}")
            ss = smpool.tile([128, 1], F32, tag="ss", name=f"ss{sc}_{hh}")
            nc.scalar.activation(sq_t[:], qk[:, o:o + 128], AF.Square,
                                 accum_out=ss[:])
            rs = smpool.tile([128, 1], F32, tag="rs", name=f"rs{sc}_{hh}")
            nc.scalar.activation(rs[:], ss[:], AF.Sqrt, bias=epsb[:],
                                 scale=1.0 / D)
            iv = smpool.tile([128, 1], F32, tag="iv", name=f"iv{sc}_{hh}")
            nc.vector.reciprocal(iv[:], rs[:])
            nc.vector.tensor_scalar_mul(qk[:, o:o + 128], qk[:, o:o + 128],
                                        iv[:])
        # rope
        rot = qkpool.tile([128, 384], F32, tag="rot", name=f"rot{sc}")
        for hh in range(3):
            o = hh * 128
            nc.vector.tensor_scalar_mul(rot[:, o:o + 64],
                                        qk[:, o + 64:o + 128], -1.0)
            nc.vector.tensor_copy(rot[:, o + 64:o + 128], qk[:, o:o + 64])
        qkr = qkpool.tile([128, 384], F32, tag="qkr", name=f"qkr{sc}")
        nc.vector.tensor_mul(qkr[:], qk[:], csb3[:, sc, :])
        nc.vector.tensor_mul(rot[:], rot[:], snb3[:, sc, :])
        nc.vector.tensor_add(qkr[:], qkr[:], rot[:])
        return qkr

    def emit_qtrans(sc, qkr):
        dests = [QT[0], QT[1], KT]
        for hh in range(3):
            pt = psC.tile([128, 128], F32, tag="psC", name=f"ptq{sc}_{hh}")
            nc.tensor.transpose(pt[:], qkr[:, hh * 128:(hh + 1) * 128],
                                idtf[:])
            if hh == 1:
                nc.vector.tensor_copy(dests[hh][:, sc * 128:(sc + 1) * 128],
                                      pt[:])
            else:
                nc.scalar.copy(dests[hh][:, sc * 128:(sc + 1) * 128], pt[:])

    # ---- Phase 3 helpers: attention, transposed orientation ----
    def emit_scores(h, t, kc):
        jlo = max(0, kc - 4 * t)
        jhi = min(3, kc + 8 - 4 * t)
        a, b = jlo * 128, (jhi + 1) * 128
        qa, qb = t * 512 + a, t * 512 + b
        sp = psA.tile([128, 512], F32, tag="psA", name=f"sp{h}_{t}_{kc}")
        nc.tensor.matmul(sp[:, a:b], KT[:, kc * 128:(kc + 1) * 128],
                         QT[h][:, qa:qb], start=True, stop=True)
        return sp, a, b

    def emit_mask_exp(h, t, kc, sp, a, b):
        j = kc - 4 * t
        if 0 <= j < 4:
            nc.vector.tensor_add(sp[:, j * 128:(j + 1) * 128],
                                 sp[:, j * 128:(j + 1) * 128], md[:])
        j2 = kc + 8 - 4 * t
        if 0 <= j2 < 4:
            nc.vector.tensor_add(sp[:, j2 * 128:(j2 + 1) * 128],
                                 sp[:, j2 * 128:(j2 + 1) * 128], me[:])
        es = espool.tile([128, 512], BF16, tag="es", name=f"es{h}_{t}_{kc}")
        nc.scalar.activation(es[:, a:b], sp[:, a:b], AF.Exp, scale=SCALE)
        return es, a, b

    # Early front-ends (score+mask+exp) for attention groups whose Q/K
    # chunks are already transposed, interleaved into the projection
    # phase: the projections keep PE busy while ACT/DVE (idle-ish there)
    # pre-compute the exp tiles, so the attention phase runs at pure
    # matmul pace.
    pre_es = {}

    def kcs_of(t):
        return list(range(max(0, 4 * t - 8), 4 * (t + 1)))

    def emit_front(h, t):
        for kc in kcs_of(t):
            sp, a, b = emit_scores(h, t, kc)
            pre_es[(h, t, kc)] = emit_mask_exp(h, t, kc, sp, a, b)

    qkr_prev = None
    for sc in range(NSC):
        qkr_cur = emit_proj(sc)
        if qkr_prev is not None:
            emit_qtrans(sc - 1, qkr_prev)
        qkr_prev = qkr_cur
    nc.scalar.activation(esink[:], sks[:], AF.Exp)
    emit_front(0, 0)
    emit_qtrans(NSC - 1, qkr_prev)
    emit_front(1, 0)

    def emit_wo(t):
        for jc in range(16):
            po = psC.tile([128, 512], F32, tag="psC", name=f"po{jc}_{t}")
            for ic in range(2):
                nc.tensor.matmul(
                    po[:], woTs[ic][:, jc * 128:(jc + 1) * 128],
                    attnT[ic][:, t * 512:(t + 1) * 512],
                    start=(ic == 0), stop=(ic == 1))
            ot = otpool.tile([128, 512], BF16, tag="ot", name=f"ot{jc}_{t}")
            if (jc + t) % 2 == 0:
                nc.vector.tensor_copy(ot[:], po[:])
            else:
                nc.scalar.copy(ot[:], po[:])
            nc.sync.dma_start(outT[jc * 128:(jc + 1) * 128,
                                   t * 512:(t + 1) * 512], ot[:])

    groups = []
    for t in range(4):
        for h in range(2):
            groups.append((h, t, list(range(max(0, 4 * t - 8), 4 * (t + 1)))))

    FRONTED = {(0, 0), (1, 0)}
    all_work = []  # flat list of (h, t, kc) still needing scores
    for h, t, kcs in groups:
        if (h, t) in FRONTED:
            continue
        for kc in kcs:
            all_work.append((h, t, kc))

    pending = {}  # (h,t,kc) -> (sp, a, b)
    LOOKAHEAD = 5
    wi = 0  # next work item to prefetch

    def prefetch(upto):
        nonlocal wi
        while wi < len(all_work) and wi < upto:
            hh, tt, kk = all_work[wi]
            pending[(hh, tt, kk)] = emit_scores(hh, tt, kk)
            wi += 1

    idx = 0
    for gi, (h, t, kcs) in enumerate(groups):
        op = psB.tile([128, 512], F32, tag="psB", name=f"op{h}_{t}")
        dp = psB.tile([1, 512], F32, tag="psB", name=f"dp{h}_{t}")
        for i, kc in enumerate(kcs):
            prefetch(idx + LOOKAHEAD)
            if (h, t, kc) in pre_es:
                es, a, b = pre_es.pop((h, t, kc))
            else:
                sp, a, b = pending.pop((h, t, kc))
                idx += 1
                es, a, b = emit_mask_exp(h, t, kc, sp, a, b)
            first, last = (i == 0), (i == len(kcs) - 1)
            nc.tensor.matmul(dp[:, a:b], ones[:], es[:, a:b],
                             start=first, stop=last)
            nc.tensor.matmul(op[:, a:b],
                             Vb[:, kc * 128:(kc + 1) * 128],
                             es[:, a:b], start=first, stop=last)
        dn = dnpool.tile([1, 512], F32, tag="dn", name=f"dn{h}_{t}")
        nc.vector.tensor_scalar_add(dn[:], dp[:], esink[:, h:h + 1])
        nc.vector.reciprocal(dn[:], dn[:])
        db = dbpool.tile([128, 512], F32, tag="db", name=f"db{h}_{t}")
        nc.gpsimd.partition_broadcast(db[:], dn[:])
        nc.vector.tensor_mul(attnT[h][:, t * 512:(t + 1) * 512], op[:],
                             db[:])
        if gi >= 2 and gi % 2 == 0:
            emit_wo(t - 1)
        if gi == len(groups) - 1:
            emit_wo(3)


_NC_CACHE = {}


def _get_nc():
    if "nc" not in _NC_CACHE:
        _NC_CACHE["nc"] = _build_kernel()
    return _NC_CACHE["nc"]


def kernel(x, cos, sin, wq, wk, wv, wo, sinks, q_norm_w, k_norm_w):
    x = np.asarray(x, np.float32).reshape(S, HID)
    xTh = np.ascontiguousarray(x.T).astype(NBF)
    cos = np.asarray(cos, np.float32)
    sin = np.asarray(sin, np.float32)
    wq = np.asarray(wq, np.float32)
    wk = np.asarray(wk, np.float32)
    wv = np.asarray(wv, np.float32)
    wo = np.asarray(wo, np.float32)
    sinks = np.asarray(sinks, np.float32)
    qw = np.asarray(q_norm_w, np.float32)
    kw = np.asarray(k_norm_w, np.float32)

    qwr = np.roll(qw, -64)
    kwr = np.roll(kw, -64)
    cs3 = np.ascontiguousarray(
        np.concatenate([cos * qw, cos * qw, cos * kw], axis=1)).astype(NBF)
    sn3 = np.ascontiguousarray(
        np.concatenate([sin * qwr, sin * qwr, sin * kwr], axis=1)).astype(NBF)
    kk = np.arange(128)[:, None]
    qq = np.arange(128)[None, :]
    maskd = np.where(kk <= qq, 0.0, NEG).astype(np.float32)
    maske = np.where(kk >= qq, 0.0, NEG).astype(np.float32)
    ident = np.eye(128, dtype=np.float32)

    in_maps = []
    for c in range(NCORES):
        kvh = c // 2
        wqkv = np.concatenate([wq[2 * c * 128:(2 * c + 2) * 128, :],
                               wk[kvh * 128:(kvh + 1) * 128, :],
                               wv[kvh * 128:(kvh + 1) * 128, :]], axis=0)
        wqkvT = np.ascontiguousarray(wqkv.T).astype(NBF)
        woT = np.ascontiguousarray(wo[:, c * 256:(c + 1) * 256].T).astype(NBF)
        in_maps.append(dict(
            xT=xTh, wqkvT=wqkvT, woT=woT, cs3=cs3, sn3=sn3,
            sinks2=np.ascontiguousarray(sinks[2 * c:2 * c + 2].reshape(1, 2)),
            maskd=maskd, maske=maske, identf=ident,
            ones1=np.ones((128, 1), NBF)))

    nc = _get_nc()
    res = run_bass_kernel_spmd(nc, in_maps, core_ids=list(range(NCORES)))
    total = res.results[0]["outT"].astype(np.float32)
    for c in range(1, NCORES):
        total = total + res.results[c]["outT"].astype(np.float32)
    return np.ascontiguousarray(total.T).reshape(1, S, HID)


# revision 58
# speedup vs baseline: 1.1943x; 1.0019x over previous
"""Sparse (sliding-window + sink) GQA attention on 8 NeuronCores.

Sharding: tensor-parallel over heads. Core c owns q-heads {2c, 2c+1} and
kv-head c//2. Each core computes its heads' attention and a partial
output projection (wo columns for its heads); host sums the 8 partials.

Data plane is bf16 (matmul inputs, DMA traffic); softmax and PSUM stay
f32. Attention runs in transposed orientation ST[k, q] so the P@V
contraction needs no on-chip transposes of the probability matrix; the
softmax denominator comes from a ones-vector matmul, and the final
normalization is folded into the PSUM->SBUF eviction of the output.
The kernel returns out^T in bf16; the host upcasts, sums and
transposes back.
"""

import numpy as np
from contextlib import ExitStack

import ml_dtypes
import concourse.bass as bass
import concourse.bacc as bacc
import concourse.mybir as mybir
import concourse.tile as tile
from concourse.bass_utils import run_bass_kernel_spmd

S = 2048
H = 16
KVH = 4
D = 128
HID = H * D
WIN = 1024
EPS = 1e-5
NCORES = 8
F32 = mybir.dt.float32
BF16 = mybir.dt.bfloat16
AF = mybir.ActivationFunctionType
SCALE = 1.0 / float(np.sqrt(D))
NEG = -1e9
NSC = S // 128  # 16 s-chunks
NEC = HID // 128  # 16 e-chunks
NBF = ml_dtypes.bfloat16


def _build_kernel():
    nc = bacc.Bacc("TRN2", target_bir_lowering=False, debug=False)

    xT = nc.dram_tensor("xT", [HID, S], BF16, kind="ExternalInput").ap()
    wqkvT = nc.dram_tensor("wqkvT", [HID, 512], BF16, kind="ExternalInput").ap()
    woT = nc.dram_tensor("woT", [256, HID], BF16, kind="ExternalInput").ap()
    cs3 = nc.dram_tensor("cs3", [S, 384], BF16, kind="ExternalInput").ap()
    sn3 = nc.dram_tensor("sn3", [S, 384], BF16, kind="ExternalInput").ap()
    sinks2 = nc.dram_tensor("sinks2", [1, 2], F32, kind="ExternalInput").ap()
    maskd = nc.dram_tensor("maskd", [128, 128], F32, kind="ExternalInput").ap()
    maske = nc.dram_tensor("maske", [128, 128], F32, kind="ExternalInput").ap()
    ones1 = nc.dram_tensor("ones1", [128, 1], BF16, kind="ExternalInput").ap()
    identf = nc.dram_tensor("identf", [128, 128], F32, kind="ExternalInput").ap()
    outT = nc.dram_tensor("outT", [HID, S], BF16, kind="ExternalOutput").ap()

    with tile.TileContext(nc) as tc:
        with ExitStack() as ctx:
            _emit(ctx, tc, nc, xT, wqkvT, woT, cs3, sn3, sinks2,
                  maskd, maske, ones1, identf, outT)
    nc.compile()
    return nc


def _emit(ctx, tc, nc, xT, wqkvT, woT, cs3, sn3, sinks2, maskd, maske,
          ones1, identf, outT):
    # persistent tensors
    pers = ctx.enter_context(tc.tile_pool(name="pers", bufs=1))
    # streaming pools
    qkpool = ctx.enter_context(tc.tile_pool(name="qk", bufs=4))
    smpool = ctx.enter_context(tc.tile_pool(name="small", bufs=6))
    espool = ctx.enter_context(tc.tile_pool(name="es", bufs=10))
    dnpool = ctx.enter_context(tc.tile_pool(name="dn", bufs=2))
    dbpool = ctx.enter_context(tc.tile_pool(name="db", bufs=2))
    otpool = ctx.enter_context(tc.tile_pool(name="ot", bufs=4))
    # psum pools
    psA = ctx.enter_context(tc.tile_pool(name="psA", bufs=3, space="PSUM"))
    psB = ctx.enter_context(tc.tile_pool(name="psB", bufs=3, space="PSUM"))
    psC = ctx.enter_context(tc.tile_pool(name="psC", bufs=2, space="PSUM"))

    QT = [pers.tile([128, S], BF16, tag=f"QT{h}", name=f"QT{h}") for h in range(2)]
    KT = pers.tile([128, S], BF16, tag="KT")
    Vb = pers.tile([128, S], BF16, tag="Vb")
    attnT = [pers.tile([128, S], BF16, tag=f"attnT{h}", name=f"attnT{h}") for h in range(2)]
    woTs = [pers.tile([128, S], BF16, tag=f"woT{i}", name=f"woT{i}") for i in range(2)]
    md = pers.tile([128, 128], F32, tag="maskd")
    me = pers.tile([128, 128], F32, tag="maske")
    idtf = pers.tile([128, 128], F32, tag="identf")
    ones = pers.tile([128, 1], BF16, tag="ones")
    sks = pers.tile([1, 2], F32, tag="sinks")
    epsb = pers.tile([128, 1], F32, tag="epsb")
    esink = pers.tile([1, 2], F32, tag="esink")

    # x blocks: 8 blocks of 256 seq, each [128 hid-part, 16 ec, 256 seq]
    xb = [pers.tile([128, NEC * 256], BF16, tag=f"xb{b}", name=f"xb{b}")
          for b in range(8)]
    xb3 = [t[:].rearrange("p (n d) -> p n d", n=NEC) for t in xb]
    wb = pers.tile([128, NEC * 512], BF16, tag="wb")
    wb3 = wb[:].rearrange("p (n d) -> p n d", n=NEC)
    csb = pers.tile([128, NSC * 384], BF16, tag="csb")
    csb3 = csb[:].rearrange("p (n d) -> p n d", n=NSC)
    snb = pers.tile([128, NSC * 384], BF16, tag="snb")
    snb3 = snb[:].rearrange("p (n d) -> p n d", n=NSC)

    nc.vector.memset(epsb[:], EPS)

    # ---- Phase 1: DMA loads (weights first: they gate the first chain) ----
    xT3 = xT.rearrange("(n p) s -> p n s", p=128)
    wq3 = wqkvT.rearrange("(n p) d -> p n d", p=128)
    cs3r = cs3.rearrange("(n p) d -> p n d", p=128)
    sn3r = sn3.rearrange("(n p) d -> p n d", p=128)
    nc.sync.dma_start(wb3[:, 0:1, :], wq3[:, 0:1, :])
    nc.sync.dma_start(xb3[0][:, 0:1, :], xT3[:, 0:1, 0:256])
    nc.sync.dma_start(wb3[:, 1:4, :], wq3[:, 1:4, :])
    nc.sync.dma_start(xb3[0][:, 1:8, :], xT3[:, 1:8, 0:256])
    nc.sync.dma_start(wb3[:, 4:8, :], wq3[:, 4:8, :])
    nc.sync.dma_start(xb3[0][:, 8:16, :], xT3[:, 8:16, 0:256])
    nc.sync.dma_start(wb3[:, 8:12, :], wq3[:, 8:12, :])
    nc.sync.dma_start(wb3[:, 12:16, :], wq3[:, 12:16, :])
    nc.sync.dma_start(idtf[:], identf[:])
    nc.sync.dma_start(csb3[:, 0:4, :], cs3r[:, 0:4, :])
    nc.sync.dma_start(snb3[:, 0:4, :], sn3r[:, 0:4, :])
    for b in range(1, 8):
        nc.sync.dma_start(xb3[b][:], xT3[:, :, b * 256:(b + 1) * 256])
        if b == 2:
            nc.sync.dma_start(csb3[:, 4:8, :], cs3r[:, 4:8, :])
            nc.sync.dma_start(snb3[:, 4:8, :], sn3r[:, 4:8, :])
            nc.sync.dma_start(md[:], maskd[:])
            nc.sync.dma_start(me[:], maske[:])
            nc.sync.dma_start(sks[:], sinks2[:])
            nc.sync.dma_start(ones[:], ones1[:])
        if b == 4:
            nc.sync.dma_start(csb3[:, 8:16, :], cs3r[:, 8:16, :])
            nc.sync.dma_start(snb3[:, 8:16, :], sn3r[:, 8:16, :])
    for i in range(2):
        nc.sync.dma_start(woTs[i][:], woT[i * 128:(i + 1) * 128, :])

    # ---- Phase 2: project, norm, rope, transpose ----
    def emit_proj(sc):
        b, lo = sc // 2, (sc % 2) * 128
        qkvp = psB.tile([128, 512], F32, tag="psB", name=f"qkvp{sc}")
        for ec in range(NEC):
            nc.tensor.matmul(qkvp[:], xb3[b][:, ec, lo:lo + 128],
                             wb3[:, ec, :],
                             start=(ec == 0), stop=(ec == NEC - 1))
        # PSUM evictions: ACT/DVE only (GPSIMD cannot access PSUM)
        qk = qkpool.tile([128, 384], BF16, tag="qk", name=f"qk{sc}")
        nc.scalar.copy(qk[:, 0:256], qkvp[:, 0:256])
        nc.vector.tensor_copy(qk[:, 256:384], qkvp[:, 256:384])
        nc.scalar.copy(Vb[:, sc * 128:(sc + 1) * 128], qkvp[:, 384:512])
        # rms norm (q0, q1, k)
        for hh in range(3):
            o = hh * 128
            sq_t = smpool.tile([128, 128], F32, tag="sq", name=f"sqr{sc}_{hh}")
            ss = smpool.tile([128, 1], F32, tag="ss", name=f"ss{sc}_{hh}")
            nc.scalar.activation(sq_t[:], qk[:, o:o + 128], AF.Square,
                                 accum_out=ss[:])
            rs = smpool.tile([128, 1], F32, tag="rs", name=f"rs{sc}_{hh}")
            nc.scalar.activation(rs[:], ss[:], AF.Sqrt, bias=epsb[:],
                                 scale=1.0 / D)
            iv = smpool.tile([128, 1], F32, tag="iv", name=f"iv{sc}_{hh}")
            nc.vector.reciprocal(iv[:], rs[:])
            nc.vector.tensor_scalar_mul(qk[:, o:o + 128], qk[:, o:o + 128],
                                        iv[:])
        # rope
        rot = qkpool.tile([128, 384], F32, tag="rot", name=f"rot{sc}")
        for hh in range(3):
            o = hh * 128
            nc.vector.tensor_scalar_mul(rot[:, o:o + 64],
                                        qk[:, o + 64:o + 128], -1.0)
            nc.vector.tensor_copy(rot[:, o + 64:o + 128], qk[:, o:o + 64])
        qkr = qkpool.tile([128, 384], F32, tag="qkr", name=f"qkr{sc}")
        nc.vector.tensor_mul(qkr[:], qk[:], csb3[:, sc, :])
        nc.vector.tensor_mul(rot[:], rot[:], snb3[:, sc, :])
        nc.vector.tensor_add(qkr[:], qkr[:], rot[:])
        return qkr

    def emit_qtrans(sc, qkr):
        dests = [QT[0], QT[1], KT]
        for hh in range(3):
            pt = psC.tile([128, 128], F32, tag="psC", name=f"ptq{sc}_{hh}")
            nc.tensor.transpose(pt[:], qkr[:, hh * 128:(hh + 1) * 128],
                                idtf[:])
            if hh == 1:
                nc.vector.tensor_copy(dests[hh][:, sc * 128:(sc + 1) * 128],
                                      pt[:])
            else:
                nc.scalar.copy(dests[hh][:, sc * 128:(sc + 1) * 128], pt[:])

    # ---- Phase 3 helpers: attention, transposed orientation ----
    def emit_scores(h, t, kc):
        jlo = max(0, kc - 4 * t)
        jhi = min(3, kc + 8 - 4 * t)
        a, b = jlo * 128, (jhi + 1) * 128
        qa, qb = t * 512 + a, t * 512 + b
        sp = psA.tile([128, 512], F32, tag="psA", name=f"sp{h}_{t}_{kc}")
        nc.tensor.matmul(sp[:, a:b], KT[:, kc * 128:(kc + 1) * 128],
                         QT[h][:, qa:qb], start=True, stop=True)
        return sp, a, b

    def emit_mask_exp(h, t, kc, sp, a, b):
        j = kc - 4 * t
        if 0 <= j < 4:
            nc.vector.tensor_add(sp[:, j * 128:(j + 1) * 128],
                                 sp[:, j * 128:(j + 1) * 128], md[:])
        j2 = kc + 8 - 4 * t
        if 0 <= j2 < 4:
            nc.vector.tensor_add(sp[:, j2 * 128:(j2 + 1) * 128],
                                 sp[:, j2 * 128:(j2 + 1) * 128], me[:])
        es = espool.tile([128, 512], BF16, tag="es", name=f"es{h}_{t}_{kc}")
        nc.scalar.activation(es[:, a:b], sp[:, a:b], AF.Exp, scale=SCALE)
        return es, a, b

    # Early front-ends (score+mask+exp) for attention groups whose Q/K
    # chunks are already transposed, interleaved into the projection
    # phase: the projections keep PE busy while ACT/DVE (idle-ish there)
    # pre-compute the exp tiles, so the attention phase runs at pure
    # matmul pace.
    pre_es = {}

    def kcs_of(t):
        return list(range(max(0, 4 * t - 8), 4 * (t + 1)))

    def emit_front(h, t):
        for kc in kcs_of(t):
            sp, a, b = emit_scores(h, t, kc)
            pre_es[(h, t, kc)] = emit_mask_exp(h, t, kc, sp, a, b)

    qkr_prev = None
    for sc in range(NSC):
        qkr_cur = emit_proj(sc)
        if qkr_prev is not None:
            emit_qtrans(sc - 1, qkr_prev)
        qkr_prev = qkr_cur
    nc.scalar.activation(esink[:], sks[:], AF.Exp)
    emit_front(0, 0)
    emit_qtrans(NSC - 1, qkr_prev)
    emit_front(1, 0)

    def emit_wo(t):
        for jc in range(16):
            po = psC.tile([128, 512], F32, tag="psC", name=f"po{jc}_{t}")
            for ic in range(2):
                nc.tensor.matmul(
                    po[:], woTs[ic][:, jc * 128:(jc + 1) * 128],
                    attnT[ic][:, t * 512:(t + 1) * 512],
                    start=(ic == 0), stop=(ic == 1))
            ot = otpool.tile([128, 512], BF16, tag="ot", name=f"ot{jc}_{t}")
            if (jc + t) % 2 == 0:
                nc.vector.tensor_copy(ot[:], po[:])
            else:
                nc.scalar.copy(ot[:], po[:])
            nc.sync.dma_start(outT[jc * 128:(jc + 1) * 128,
                                   t * 512:(t + 1) * 512], ot[:])

    groups = []
    for t in range(4):
        for h in range(2):
            groups.append((h, t, list(range(max(0, 4 * t - 8), 4 * (t + 1)))))

    FRONTED = {(0, 0), (1, 0)}
    all_work = []  # flat list of (h, t, kc) still needing scores
    for h, t, kcs in groups:
        if (h, t) in FRONTED:
            continue
        for kc in kcs:
            all_work.append((h, t, kc))

    pending = {}  # (h,t,kc) -> (sp, a, b)
    LOOKAHEAD = 7
    wi = 0  # next work item to prefetch

    def prefetch(upto):
        nonlocal wi
        while wi < len(all_work) and wi < upto:
            hh, tt, kk = all_work[wi]
            pending[(hh, tt, kk)] = emit_scores(hh, tt, kk)
            wi += 1

    idx = 0
    for gi, (h, t, kcs) in enumerate(groups):
        op = psB.tile([128, 512], F32, tag="psB", name=f"op{h}_{t}")
        dp = psB.tile([1, 512], F32, tag="psB", name=f"dp{h}_{t}")
        for i, kc in enumerate(kcs):
            prefetch(idx + LOOKAHEAD)
            if (h, t, kc) in pre_es:
                es, a, b = pre_es.pop((h, t, kc))
            else:
                sp, a, b = pending.pop((h, t, kc))
                idx += 1
                es, a, b = emit_mask_exp(h, t, kc, sp, a, b)
            first, last = (i == 0), (i == len(kcs) - 1)
            nc.tensor.matmul(dp[:, a:b], ones[:], es[:, a:b],
                             start=first, stop=last)
            nc.tensor.matmul(op[:, a:b],
                             Vb[:, kc * 128:(kc + 1) * 128],
                             es[:, a:b], start=first, stop=last)
        dn = dnpool.tile([1, 512], F32, tag="dn", name=f"dn{h}_{t}")
        nc.vector.tensor_scalar_add(dn[:], dp[:], esink[:, h:h + 1])
        nc.vector.reciprocal(dn[:], dn[:])
        db = dbpool.tile([128, 512], F32, tag="db", name=f"db{h}_{t}")
        nc.gpsimd.partition_broadcast(db[:], dn[:])
        nc.vector.tensor_mul(attnT[h][:, t * 512:(t + 1) * 512], op[:],
                             db[:])
        if gi >= 2 and gi % 2 == 0:
            emit_wo(t - 1)
        if gi == len(groups) - 1:
            emit_wo(3)


_NC_CACHE = {}


def _get_nc():
    if "nc" not in _NC_CACHE:
        _NC_CACHE["nc"] = _build_kernel()
    return _NC_CACHE["nc"]


def kernel(x, cos, sin, wq, wk, wv, wo, sinks, q_norm_w, k_norm_w):
    x = np.asarray(x, np.float32).reshape(S, HID)
    xTh = np.ascontiguousarray(x.T).astype(NBF)
    cos = np.asarray(cos, np.float32)
    sin = np.asarray(sin, np.float32)
    wq = np.asarray(wq, np.float32)
    wk = np.asarray(wk, np.float32)
    wv = np.asarray(wv, np.float32)
    wo = np.asarray(wo, np.float32)
    sinks = np.asarray(sinks, np.float32)
    qw = np.asarray(q_norm_w, np.float32)
    kw = np.asarray(k_norm_w, np.float32)

    qwr = np.roll(qw, -64)
    kwr = np.roll(kw, -64)
    cs3 = np.ascontiguousarray(
        np.concatenate([cos * qw, cos * qw, cos * kw], axis=1)).astype(NBF)
    sn3 = np.ascontiguousarray(
        np.concatenate([sin * qwr, sin * qwr, sin * kwr], axis=1)).astype(NBF)
    kk = np.arange(128)[:, None]
    qq = np.arange(128)[None, :]
    maskd = np.where(kk <= qq, 0.0, NEG).astype(np.float32)
    maske = np.where(kk >= qq, 0.0, NEG).astype(np.float32)
    ident = np.eye(128, dtype=np.float32)

    in_maps = []
    for c in range(NCORES):
        kvh = c // 2
        wqkv = np.concatenate([wq[2 * c * 128:(2 * c + 2) * 128, :],
                               wk[kvh * 128:(kvh + 1) * 128, :],
                               wv[kvh * 128:(kvh + 1) * 128, :]], axis=0)
        wqkvT = np.ascontiguousarray(wqkv.T).astype(NBF)
        woT = np.ascontiguousarray(wo[:, c * 256:(c + 1) * 256].T).astype(NBF)
        in_maps.append(dict(
            xT=xTh, wqkvT=wqkvT, woT=woT, cs3=cs3, sn3=sn3,
            sinks2=np.ascontiguousarray(sinks[2 * c:2 * c + 2].reshape(1, 2)),
            maskd=maskd, maske=maske, identf=ident,
            ones1=np.ones((128, 1), NBF)))

    nc = _get_nc()
    res = run_bass_kernel_spmd(nc, in_maps, core_ids=list(range(NCORES)))
    total = res.results[0]["outT"].astype(np.float32)
    for c in range(1, NCORES):
        total = total + res.results[c]["outT"].astype(np.float32)
    return np.ascontiguousarray(total.T).reshape(1, S, HID)


# revision 65
# speedup vs baseline: 1.2251x; 1.0258x over previous
"""Sparse (sliding-window + sink) GQA attention on 8 NeuronCores.

Sharding: tensor-parallel over heads. Core c owns q-heads {2c, 2c+1} and
kv-head c//2. Each core computes its heads' attention and a partial
output projection (wo columns for its heads); host sums the 8 partials.

Data plane is bf16 (matmul inputs, DMA traffic); softmax and PSUM stay
f32. Attention runs in transposed orientation ST[k, q] so the P@V
contraction needs no on-chip transposes of the probability matrix; the
softmax denominator comes from a ones-vector matmul, and the final
normalization is folded into the PSUM->SBUF eviction of the output.
The kernel returns out^T in bf16; the host upcasts, sums and
transposes back.
"""

import numpy as np
from contextlib import ExitStack

import ml_dtypes
import concourse.bass as bass
import concourse.bacc as bacc
import concourse.mybir as mybir
import concourse.tile as tile
from concourse.bass_utils import run_bass_kernel_spmd

S = 2048
H = 16
KVH = 4
D = 128
HID = H * D
WIN = 1024
EPS = 1e-5
NCORES = 8
F32 = mybir.dt.float32
BF16 = mybir.dt.bfloat16
AF = mybir.ActivationFunctionType
SCALE = 1.0 / float(np.sqrt(D))
NEG = -1e9
NSC = S // 128  # 16 s-chunks
NEC = HID // 128  # 16 e-chunks
NBF = ml_dtypes.bfloat16


def _build_kernel():
    nc = bacc.Bacc("TRN2", target_bir_lowering=False, debug=False)

    xT = nc.dram_tensor("xT", [HID, S], BF16, kind="ExternalInput").ap()
    wqkvT = nc.dram_tensor("wqkvT", [HID, 512], BF16, kind="ExternalInput").ap()
    woT = nc.dram_tensor("woT", [256, HID], BF16, kind="ExternalInput").ap()
    cs3 = nc.dram_tensor("cs3", [S, 384], BF16, kind="ExternalInput").ap()
    sn3 = nc.dram_tensor("sn3", [S, 384], BF16, kind="ExternalInput").ap()
    sinks2 = nc.dram_tensor("sinks2", [1, 2], F32, kind="ExternalInput").ap()
    maskd = nc.dram_tensor("maskd", [128, 128], F32, kind="ExternalInput").ap()
    maske = nc.dram_tensor("maske", [128, 128], F32, kind="ExternalInput").ap()
    ones1 = nc.dram_tensor("ones1", [128, 1], BF16, kind="ExternalInput").ap()
    identf = nc.dram_tensor("identf", [128, 128], F32, kind="ExternalInput").ap()
    outT = nc.dram_tensor("outT", [HID, S], BF16, kind="ExternalOutput").ap()

    with tile.TileContext(nc) as tc:
        with ExitStack() as ctx:
            _emit(ctx, tc, nc, xT, wqkvT, woT, cs3, sn3, sinks2,
                  maskd, maske, ones1, identf, outT)
    nc.compile()
    return nc


def _emit(ctx, tc, nc, xT, wqkvT, woT, cs3, sn3, sinks2, maskd, maske,
          ones1, identf, outT):
    # persistent tensors
    pers = ctx.enter_context(tc.tile_pool(name="pers", bufs=1))
    # streaming pools
    qkpool = ctx.enter_context(tc.tile_pool(name="qk", bufs=4))
    smpool = ctx.enter_context(tc.tile_pool(name="small", bufs=6))
    espool = ctx.enter_context(tc.tile_pool(name="es", bufs=10))
    dnpool = ctx.enter_context(tc.tile_pool(name="dn", bufs=2))
    dbpool = ctx.enter_context(tc.tile_pool(name="db", bufs=2))
    otpool = ctx.enter_context(tc.tile_pool(name="ot", bufs=4))
    # psum pools
    psA = ctx.enter_context(tc.tile_pool(name="psA", bufs=3, space="PSUM"))
    psB = ctx.enter_context(tc.tile_pool(name="psB", bufs=3, space="PSUM"))
    psC = ctx.enter_context(tc.tile_pool(name="psC", bufs=2, space="PSUM"))

    QT = [pers.tile([128, S], BF16, tag=f"QT{h}", name=f"QT{h}") for h in range(2)]
    KT = pers.tile([128, S], BF16, tag="KT")
    Vb = pers.tile([128, S], BF16, tag="Vb")
    attnT = [pers.tile([128, S], BF16, tag=f"attnT{h}", name=f"attnT{h}") for h in range(2)]
    woTs = [pers.tile([128, S], BF16, tag=f"woT{i}", name=f"woT{i}") for i in range(2)]
    md = pers.tile([128, 128], F32, tag="maskd")
    me = pers.tile([128, 128], F32, tag="maske")
    idtf = pers.tile([128, 128], F32, tag="identf")
    ones = pers.tile([128, 1], BF16, tag="ones")
    sks = pers.tile([1, 2], F32, tag="sinks")
    epsb = pers.tile([128, 1], F32, tag="epsb")

    # x blocks: 8 blocks of 256 seq, each [128 hid-part, 16 ec, 256 seq]
    xb = [pers.tile([128, NEC * 256], BF16, tag=f"xb{b}", name=f"xb{b}")
          for b in range(8)]
    xb3 = [t[:].rearrange("p (n d) -> p n d", n=NEC) for t in xb]
    wb = pers.tile([128, NEC * 512], BF16, tag="wb")
    wb3 = wb[:].rearrange("p (n d) -> p n d", n=NEC)
    csb = pers.tile([128, NSC * 384], BF16, tag="csb")
    csb3 = csb[:].rearrange("p (n d) -> p n d", n=NSC)
    snb = pers.tile([128, NSC * 384], BF16, tag="snb")
    snb3 = snb[:].rearrange("p (n d) -> p n d", n=NSC)

    nc.vector.memset(epsb[:], EPS)
    # Dummy Sqrt as the very first activation: the table-load pass then
    # loads the sqrt_and_others set once up front (it also covers the
    # Copy/Square the projection phase uses), instead of switching
    # tables mid-projection and stalling the norm chain.
    dum = pers.tile([128, 1], F32, tag="dum")
    nc.scalar.activation(dum[:], epsb[:], AF.Sqrt)

    # ---- Phase 1: DMA loads (weights first: they gate the first chain) ----
    xT3 = xT.rearrange("(n p) s -> p n s", p=128)
    wq3 = wqkvT.rearrange("(n p) d -> p n d", p=128)
    cs3r = cs3.rearrange("(n p) d -> p n d", p=128)
    sn3r = sn3.rearrange("(n p) d -> p n d", p=128)
    nc.sync.dma_start(wb3[:, 0:1, :], wq3[:, 0:1, :])
    nc.sync.dma_start(xb3[0][:, 0:1, :], xT3[:, 0:1, 0:256])
    nc.sync.dma_start(wb3[:, 1:4, :], wq3[:, 1:4, :])
    nc.sync.dma_start(xb3[0][:, 1:8, :], xT3[:, 1:8, 0:256])
    nc.sync.dma_start(wb3[:, 4:8, :], wq3[:, 4:8, :])
    nc.sync.dma_start(xb3[0][:, 8:16, :], xT3[:, 8:16, 0:256])
    nc.sync.dma_start(wb3[:, 8:12, :], wq3[:, 8:12, :])
    nc.sync.dma_start(wb3[:, 12:16, :], wq3[:, 12:16, :])
    nc.sync.dma_start(idtf[:], identf[:])
    nc.sync.dma_start(csb3[:, 0:4, :], cs3r[:, 0:4, :])
    nc.sync.dma_start(snb3[:, 0:4, :], sn3r[:, 0:4, :])
    for b in range(1, 8):
        nc.sync.dma_start(xb3[b][:], xT3[:, :, b * 256:(b + 1) * 256])
        if b == 2:
            nc.sync.dma_start(csb3[:, 4:8, :], cs3r[:, 4:8, :])
            nc.sync.dma_start(snb3[:, 4:8, :], sn3r[:, 4:8, :])
            nc.sync.dma_start(md[:], maskd[:])
            nc.sync.dma_start(me[:], maske[:])
            nc.sync.dma_start(sks[:], sinks2[:])
            nc.sync.dma_start(ones[:], ones1[:])
        if b == 4:
            nc.sync.dma_start(csb3[:, 8:16, :], cs3r[:, 8:16, :])
            nc.sync.dma_start(snb3[:, 8:16, :], sn3r[:, 8:16, :])
    for i in range(2):
        nc.sync.dma_start(woTs[i][:], woT[i * 128:(i + 1) * 128, :])

    # ---- Phase 2: project, norm, rope, transpose ----
    def emit_proj(sc):
        b, lo = sc // 2, (sc % 2) * 128
        qkvp = psB.tile([128, 512], F32, tag="psB", name=f"qkvp{sc}")
        for ec in range(NEC):
            nc.tensor.matmul(qkvp[:], xb3[b][:, ec, lo:lo + 128],
                             wb3[:, ec, :],
                             start=(ec == 0), stop=(ec == NEC - 1))
        # PSUM evictions: ACT/DVE only (GPSIMD cannot access PSUM)
        qk = qkpool.tile([128, 384], BF16, tag="qk", name=f"qk{sc}")
        nc.scalar.copy(qk[:, 0:256], qkvp[:, 0:256])
        nc.vector.tensor_copy(qk[:, 256:384], qkvp[:, 256:384])
        nc.scalar.copy(Vb[:, sc * 128:(sc + 1) * 128], qkvp[:, 384:512])
        # rms norm (q0, q1, k)
        for hh in range(3):
            o = hh * 128
            sq_t = smpool.tile([128, 128], F32, tag="sq", name=f"sqr{sc}_{hh}")
            ss = smpool.tile([128, 1], F32, tag="ss", name=f"ss{sc}_{hh}")
            nc.scalar.activation(sq_t[:], qk[:, o:o + 128], AF.Square,
                                 accum_out=ss[:])
            rs = smpool.tile([128, 1], F32, tag="rs", name=f"rs{sc}_{hh}")
            nc.scalar.activation(rs[:], ss[:], AF.Sqrt, bias=epsb[:],
                                 scale=1.0 / D)
            iv = smpool.tile([128, 1], F32, tag="iv", name=f"iv{sc}_{hh}")
            nc.vector.reciprocal(iv[:], rs[:])
            nc.vector.tensor_scalar_mul(qk[:, o:o + 128], qk[:, o:o + 128],
                                        iv[:])
        # rope
        rot = qkpool.tile([128, 384], F32, tag="rot", name=f"rot{sc}")
        for hh in range(3):
            o = hh * 128
            nc.vector.tensor_scalar_mul(rot[:, o:o + 64],
                                        qk[:, o + 64:o + 128], -1.0)
            nc.vector.tensor_copy(rot[:, o + 64:o + 128], qk[:, o:o + 64])
        qkr = qkpool.tile([128, 384], F32, tag="qkr", name=f"qkr{sc}")
        nc.vector.tensor_mul(qkr[:], qk[:], csb3[:, sc, :])
        nc.vector.tensor_mul(rot[:], rot[:], snb3[:, sc, :])
        nc.vector.tensor_add(qkr[:], qkr[:], rot[:])
        return qkr

    def emit_qtrans(sc, qkr):
        dests = [QT[0], QT[1], KT]
        for hh in range(3):
            pt = psC.tile([128, 128], F32, tag="psC", name=f"ptq{sc}_{hh}")
            nc.tensor.transpose(pt[:], qkr[:, hh * 128:(hh + 1) * 128],
                                idtf[:])
            if hh == 1:
                nc.vector.tensor_copy(dests[hh][:, sc * 128:(sc + 1) * 128],
                                      pt[:])
            else:
                nc.scalar.copy(dests[hh][:, sc * 128:(sc + 1) * 128], pt[:])

    # ---- Phase 3 helpers: attention, transposed orientation ----
    def emit_scores(h, t, kc):
        jlo = max(0, kc - 4 * t)
        jhi = min(3, kc + 8 - 4 * t)
        a, b = jlo * 128, (jhi + 1) * 128
        qa, qb = t * 512 + a, t * 512 + b
        sp = psA.tile([128, 512], F32, tag="psA", name=f"sp{h}_{t}_{kc}")
        nc.tensor.matmul(sp[:, a:b], KT[:, kc * 128:(kc + 1) * 128],
                         QT[h][:, qa:qb], start=True, stop=True)
        return sp, a, b

    def emit_mask_exp(h, t, kc, sp, a, b):
        j = kc - 4 * t
        if 0 <= j < 4:
            nc.vector.tensor_add(sp[:, j * 128:(j + 1) * 128],
                                 sp[:, j * 128:(j + 1) * 128], md[:])
        j2 = kc + 8 - 4 * t
        if 0 <= j2 < 4:
            nc.vector.tensor_add(sp[:, j2 * 128:(j2 + 1) * 128],
                                 sp[:, j2 * 128:(j2 + 1) * 128], me[:])
        es = espool.tile([128, 512], BF16, tag="es", name=f"es{h}_{t}_{kc}")
        nc.scalar.activation(es[:, a:b], sp[:, a:b], AF.Exp, scale=SCALE)
        return es, a, b

    # Early front-ends (score+mask+exp) for attention groups whose Q/K
    # chunks are already transposed, interleaved into the projection
    # phase: the projections keep PE busy while ACT/DVE (idle-ish there)
    # pre-compute the exp tiles, so the attention phase runs at pure
    # matmul pace.
    pre_es = {}

    def kcs_of(t):
        return list(range(max(0, 4 * t - 8), 4 * (t + 1)))

    def emit_front(h, t):
        for kc in kcs_of(t):
            sp, a, b = emit_scores(h, t, kc)
            pre_es[(h, t, kc)] = emit_mask_exp(h, t, kc, sp, a, b)

    qkr_hist = {}
    for sc in range(NSC):
        qkr_hist[sc] = emit_proj(sc)
        if sc >= 2:
            emit_qtrans(sc - 2, qkr_hist.pop(sc - 2))
    emit_qtrans(NSC - 2, qkr_hist.pop(NSC - 2))
    emit_front(0, 0)
    emit_front(1, 0)
    qkr_last = qkr_hist.pop(NSC - 1)

    def emit_wo(t):
        for jc in range(16):
            po = psC.tile([128, 512], F32, tag="psC", name=f"po{jc}_{t}")
            for ic in range(2):
                nc.tensor.matmul(
                    po[:], woTs[ic][:, jc * 128:(jc + 1) * 128],
                    attnT[ic][:, t * 512:(t + 1) * 512],
                    start=(ic == 0), stop=(ic == 1))
            ot = otpool.tile([128, 512], BF16, tag="ot", name=f"ot{jc}_{t}")
            if (jc + t) % 2 == 0:
                nc.vector.tensor_copy(ot[:], po[:])
            else:
                nc.scalar.copy(ot[:], po[:])
            nc.sync.dma_start(outT[jc * 128:(jc + 1) * 128,
                                   t * 512:(t + 1) * 512], ot[:])

    groups = []
    for t in range(4):
        for h in range(2):
            groups.append((h, t, list(range(max(0, 4 * t - 8), 4 * (t + 1)))))

    FRONTED = {(0, 0), (1, 0)}
    all_work = []  # flat list of (h, t, kc) still needing scores
    for h, t, kcs in groups:
        if (h, t) in FRONTED:
            continue
        for kc in kcs:
            all_work.append((h, t, kc))

    pending = {}  # (h,t,kc) -> (sp, a, b)
    LOOKAHEAD = 7
    wi = 0  # next work item to prefetch

    def prefetch(upto):
        nonlocal wi
        while wi < len(all_work) and wi < upto:
            hh, tt, kk = all_work[wi]
            pending[(hh, tt, kk)] = emit_scores(hh, tt, kk)
            wi += 1

    idx = 0
    for gi, (h, t, kcs) in enumerate(groups):
        op = psB.tile([128, 512], F32, tag="psB", name=f"op{h}_{t}")
        dp = psB.tile([1, 512], F32, tag="psB", name=f"dp{h}_{t}")
        for i, kc in enumerate(kcs):
            prefetch(idx + LOOKAHEAD)
            if (h, t, kc) in pre_es:
                es, a, b = pre_es.pop((h, t, kc))
            else:
                sp, a, b = pending.pop((h, t, kc))
                idx += 1
                es, a, b = emit_mask_exp(h, t, kc, sp, a, b)
            first, last = (i == 0), (i == len(kcs) - 1)
            nc.tensor.matmul(dp[:, a:b], ones[:], es[:, a:b],
                             start=first, stop=last)
            nc.tensor.matmul(op[:, a:b],
                             Vb[:, kc * 128:(kc + 1) * 128],
                             es[:, a:b], start=first, stop=last)
        dn = dnpool.tile([1, 512], F32, tag="dn", name=f"dn{h}_{t}")
        nc.vector.tensor_scalar_add(dn[:], dp[:], sks[:, h:h + 1])
        nc.vector.reciprocal(dn[:], dn[:])
        db = dbpool.tile([128, 512], F32, tag="db", name=f"db{h}_{t}")
        nc.gpsimd.partition_broadcast(db[:], dn[:])
        nc.vector.tensor_mul(attnT[h][:, t * 512:(t + 1) * 512], op[:],
                             db[:])
        if gi == 1:
            # last s-chunk's transposes, deferred past the first groups so
            # its norm/rope chain never stalls the PE queue; results are
            # only needed by the t=3 groups much later.
            emit_qtrans(NSC - 1, qkr_last)
        if gi >= 2 and gi % 2 == 0:
            emit_wo(t - 1)
        if gi == len(groups) - 1:
            emit_wo(3)


_NC_CACHE = {}


def _get_nc():
    if "nc" not in _NC_CACHE:
        _NC_CACHE["nc"] = _build_kernel()
    return _NC_CACHE["nc"]


def kernel(x, cos, sin, wq, wk, wv, wo, sinks, q_norm_w, k_norm_w):
    x = np.asarray(x, np.float32).reshape(S, HID)
    xTh = np.ascontiguousarray(x.T).astype(NBF)
    cos = np.asarray(cos, np.float32)
    sin = np.asarray(sin, np.float32)
    wq = np.asarray(wq, np.float32)
    wk = np.asarray(wk, np.float32)
    wv = np.asarray(wv, np.float32)
    wo = np.asarray(wo, np.float32)
    sinks = np.asarray(sinks, np.float32)
    qw = np.asarray(q_norm_w, np.float32)
    kw = np.asarray(k_norm_w, np.float32)

    qwr = np.roll(qw, -64)
    kwr = np.roll(kw, -64)
    cs3 = np.ascontiguousarray(
        np.concatenate([cos * qw, cos * qw, cos * kw], axis=1)).astype(NBF)
    sn3 = np.ascontiguousarray(
        np.concatenate([sin * qwr, sin * qwr, sin * kwr], axis=1)).astype(NBF)
    kk = np.arange(128)[:, None]
    qq = np.arange(128)[None, :]
    maskd = np.where(kk <= qq, 0.0, NEG).astype(np.float32)
    maske = np.where(kk >= qq, 0.0, NEG).astype(np.float32)
    ident = np.eye(128, dtype=np.float32)

    in_maps = []
    for c in range(NCORES):
        kvh = c // 2
        wqkv = np.concatenate([wq[2 * c * 128:(2 * c + 2) * 128, :],
                               wk[kvh * 128:(kvh + 1) * 128, :],
                               wv[kvh * 128:(kvh + 1) * 128, :]], axis=0)
        wqkvT = np.ascontiguousarray(wqkv.T).astype(NBF)
        woT = np.ascontiguousarray(wo[:, c * 256:(c + 1) * 256].T).astype(NBF)
        in_maps.append(dict(
            xT=xTh, wqkvT=wqkvT, woT=woT, cs3=cs3, sn3=sn3,
            sinks2=np.ascontiguousarray(
                np.exp(sinks[2 * c:2 * c + 2]).reshape(1, 2)),
            maskd=maskd, maske=maske, identf=ident,
            ones1=np.ones((128, 1), NBF)))

    nc = _get_nc()
    res = run_bass_kernel_spmd(nc, in_maps, core_ids=list(range(NCORES)))
    total = res.results[0]["outT"].astype(np.float32)
    for c in range(1, NCORES):
        total = total + res.results[c]["outT"].astype(np.float32)
    return np.ascontiguousarray(total.T).reshape(1, S, HID)


# revision 70
# speedup vs baseline: 1.2671x; 1.0343x over previous
"""Sparse (sliding-window + sink) GQA attention on 8 NeuronCores.

Sharding: tensor-parallel over heads. Core c owns q-heads {2c, 2c+1} and
kv-head c//2. Each core computes its heads' attention and a partial
output projection (wo columns for its heads); host sums the 8 partials.

Data plane is bf16 (matmul inputs, DMA traffic); softmax and PSUM stay
f32. Attention runs in transposed orientation ST[k, q] so the P@V
contraction needs no on-chip transposes of the probability matrix; the
softmax denominator comes from a ones-vector matmul, and the final
normalization is folded into the PSUM->SBUF eviction of the output.
The kernel returns out^T in bf16; the host upcasts, sums and
transposes back.
"""

import numpy as np
from contextlib import ExitStack

import ml_dtypes
import concourse.bass as bass
import concourse.bacc as bacc
import concourse.mybir as mybir
import concourse.tile as tile
from concourse.bass_utils import run_bass_kernel_spmd

S = 2048
H = 16
KVH = 4
D = 128
HID = H * D
WIN = 1024
EPS = 1e-5
NCORES = 8
F32 = mybir.dt.float32
BF16 = mybir.dt.bfloat16
AF = mybir.ActivationFunctionType
SCALE = 1.0 / float(np.sqrt(D))
NEG = -1e9
NSC = S // 128  # 16 s-chunks
NEC = HID // 128  # 16 e-chunks
NBF = ml_dtypes.bfloat16


def _build_kernel():
    nc = bacc.Bacc("TRN2", target_bir_lowering=False, debug=False)

    xT = nc.dram_tensor("xT", [HID, S], BF16, kind="ExternalInput").ap()
    wqkvT = nc.dram_tensor("wqkvT", [HID, 512], BF16, kind="ExternalInput").ap()
    woT = nc.dram_tensor("woT", [256, HID], BF16, kind="ExternalInput").ap()
    cs3 = nc.dram_tensor("cs3", [S, 384], BF16, kind="ExternalInput").ap()
    sn3 = nc.dram_tensor("sn3", [S, 384], BF16, kind="ExternalInput").ap()
    sinks2 = nc.dram_tensor("sinks2", [1, 2], F32, kind="ExternalInput").ap()
    maskd = nc.dram_tensor("maskd", [128, 128], F32, kind="ExternalInput").ap()
    maske = nc.dram_tensor("maske", [128, 128], F32, kind="ExternalInput").ap()
    ones1 = nc.dram_tensor("ones1", [128, 1], BF16, kind="ExternalInput").ap()
    identf = nc.dram_tensor("identf", [128, 128], F32, kind="ExternalInput").ap()
    outT = nc.dram_tensor("outT", [HID, S], BF16, kind="ExternalOutput").ap()

    with tile.TileContext(nc) as tc:
        with ExitStack() as ctx:
            _emit(ctx, tc, nc, xT, wqkvT, woT, cs3, sn3, sinks2,
                  maskd, maske, ones1, identf, outT)
    nc.compile()
    return nc


def _emit(ctx, tc, nc, xT, wqkvT, woT, cs3, sn3, sinks2, maskd, maske,
          ones1, identf, outT):
    # persistent tensors
    pers = ctx.enter_context(tc.tile_pool(name="pers", bufs=1))
    # streaming pools
    qkpool = ctx.enter_context(tc.tile_pool(name="qk", bufs=6))
    smpool = ctx.enter_context(tc.tile_pool(name="small", bufs=8))
    espool = ctx.enter_context(tc.tile_pool(name="es", bufs=12))
    dnpool = ctx.enter_context(tc.tile_pool(name="dn", bufs=4))
    dbpool = ctx.enter_context(tc.tile_pool(name="db", bufs=4))
    otpool = ctx.enter_context(tc.tile_pool(name="ot", bufs=8))
    # psum pools
    psA = ctx.enter_context(tc.tile_pool(name="psA", bufs=3, space="PSUM"))
    psB = ctx.enter_context(tc.tile_pool(name="psB", bufs=3, space="PSUM"))
    psC = ctx.enter_context(tc.tile_pool(name="psC", bufs=2, space="PSUM"))

    QT = [pers.tile([128, S], BF16, tag=f"QT{h}", name=f"QT{h}") for h in range(2)]
    KT = pers.tile([128, S], BF16, tag="KT")
    Vb = pers.tile([128, S], BF16, tag="Vb")
    attnT = [pers.tile([128, S], BF16, tag=f"attnT{h}", name=f"attnT{h}") for h in range(2)]
    woTs = [pers.tile([128, S], BF16, tag=f"woT{i}", name=f"woT{i}") for i in range(2)]
    md = pers.tile([128, 128], F32, tag="maskd")
    me = pers.tile([128, 128], F32, tag="maske")
    idtf = pers.tile([128, 128], F32, tag="identf")
    ones = pers.tile([128, 1], BF16, tag="ones")
    sks = pers.tile([1, 2], F32, tag="sinks")
    epsb = pers.tile([128, 1], F32, tag="epsb")

    # x blocks: 8 blocks of 256 seq, each [128 hid-part, 16 ec, 256 seq]
    xb = [pers.tile([128, NEC * 256], BF16, tag=f"xb{b}", name=f"xb{b}")
          for b in range(8)]
    xb3 = [t[:].rearrange("p (n d) -> p n d", n=NEC) for t in xb]
    wb = pers.tile([128, NEC * 512], BF16, tag="wb")
    wb3 = wb[:].rearrange("p (n d) -> p n d", n=NEC)
    csb = pers.tile([128, NSC * 384], BF16, tag="csb")
    csb3 = csb[:].rearrange("p (n d) -> p n d", n=NSC)
    snb = pers.tile([128, NSC * 384], BF16, tag="snb")
    snb3 = snb[:].rearrange("p (n d) -> p n d", n=NSC)

    nc.vector.memset(epsb[:], EPS)
    # Dummy Sqrt as the very first activation: the table-load pass then
    # loads the sqrt_and_others set once up front (it also covers the
    # Copy/Square the projection phase uses), instead of switching
    # tables mid-projection and stalling the norm chain.
    dum = pers.tile([128, 1], F32, tag="dum")
    nc.scalar.activation(dum[:], epsb[:], AF.Sqrt)

    # ---- Phase 1: DMA loads (weights first: they gate the first chain) ----
    xT3 = xT.rearrange("(n p) s -> p n s", p=128)
    wq3 = wqkvT.rearrange("(n p) d -> p n d", p=128)
    cs3r = cs3.rearrange("(n p) d -> p n d", p=128)
    sn3r = sn3.rearrange("(n p) d -> p n d", p=128)
    nc.sync.dma_start(wb3[:, 0:1, :], wq3[:, 0:1, :])
    nc.sync.dma_start(xb3[0][:, 0:1, :], xT3[:, 0:1, 0:256])
    nc.sync.dma_start(wb3[:, 1:4, :], wq3[:, 1:4, :])
    nc.sync.dma_start(xb3[0][:, 1:8, :], xT3[:, 1:8, 0:256])
    nc.sync.dma_start(wb3[:, 4:8, :], wq3[:, 4:8, :])
    nc.sync.dma_start(xb3[0][:, 8:16, :], xT3[:, 8:16, 0:256])
    nc.sync.dma_start(wb3[:, 8:12, :], wq3[:, 8:12, :])
    nc.sync.dma_start(wb3[:, 12:16, :], wq3[:, 12:16, :])
    nc.sync.dma_start(idtf[:], identf[:])
    for b in range(1, 8):
        nc.sync.dma_start(xb3[b][:], xT3[:, :, b * 256:(b + 1) * 256])
        if b == 1:
            nc.sync.dma_start(csb3[:, 0:4, :], cs3r[:, 0:4, :])
            nc.sync.dma_start(snb3[:, 0:4, :], sn3r[:, 0:4, :])
        if b == 2:
            nc.sync.dma_start(csb3[:, 4:8, :], cs3r[:, 4:8, :])
            nc.sync.dma_start(snb3[:, 4:8, :], sn3r[:, 4:8, :])
            nc.sync.dma_start(md[:], maskd[:])
            nc.sync.dma_start(me[:], maske[:])
            nc.sync.dma_start(sks[:], sinks2[:])
            nc.sync.dma_start(ones[:], ones1[:])
        if b == 4:
            nc.sync.dma_start(csb3[:, 8:16, :], cs3r[:, 8:16, :])
            nc.sync.dma_start(snb3[:, 8:16, :], sn3r[:, 8:16, :])
    for i in range(2):
        nc.sync.dma_start(woTs[i][:], woT[i * 128:(i + 1) * 128, :])

    # ---- Phase 2: project, norm, rope, transpose ----
    def emit_proj(sc):
        b, lo = sc // 2, (sc % 2) * 128
        qkvp = psB.tile([128, 512], F32, tag="psB", name=f"qkvp{sc}")
        for ec in range(NEC):
            nc.tensor.matmul(qkvp[:], xb3[b][:, ec, lo:lo + 128],
                             wb3[:, ec, :],
                             start=(ec == 0), stop=(ec == NEC - 1))
        # PSUM evictions: ACT/DVE only (GPSIMD cannot access PSUM)
        qk = qkpool.tile([128, 384], BF16, tag="qk", name=f"qk{sc}")
        nc.scalar.copy(qk[:, 0:256], qkvp[:, 0:256])
        nc.vector.tensor_copy(qk[:, 256:384], qkvp[:, 256:384])
        nc.scalar.copy(Vb[:, sc * 128:(sc + 1) * 128], qkvp[:, 384:512])
        # rms norm (q0, q1, k)
        for hh in range(3):
            o = hh * 128
            sq_t = smpool.tile([128, 128], F32, tag="sq", name=f"sqr{sc}_{hh}")
            ss = smpool.tile([128, 1], F32, tag="ss", name=f"ss{sc}_{hh}")
            nc.scalar.activation(sq_t[:], qk[:, o:o + 128], AF.Square,
                                 accum_out=ss[:])
            rs = smpool.tile([128, 1], F32, tag="rs", name=f"rs{sc}_{hh}")
            nc.scalar.activation(rs[:], ss[:], AF.Sqrt, bias=epsb[:],
                                 scale=1.0 / D)
            iv = smpool.tile([128, 1], F32, tag="iv", name=f"iv{sc}_{hh}")
            nc.vector.reciprocal(iv[:], rs[:])
            nc.vector.tensor_scalar_mul(qk[:, o:o + 128], qk[:, o:o + 128],
                                        iv[:])
        # rope
        rot = qkpool.tile([128, 384], F32, tag="rot", name=f"rot{sc}")
        for hh in range(3):
            o = hh * 128
            nc.vector.tensor_scalar_mul(rot[:, o:o + 64],
                                        qk[:, o + 64:o + 128], -1.0)
            nc.vector.tensor_copy(rot[:, o + 64:o + 128], qk[:, o:o + 64])
        qkr = qkpool.tile([128, 384], F32, tag="qkr", name=f"qkr{sc}")
        nc.vector.tensor_mul(qkr[:], qk[:], csb3[:, sc, :])
        nc.vector.tensor_mul(rot[:], rot[:], snb3[:, sc, :])
        nc.vector.tensor_add(qkr[:], qkr[:], rot[:])
        return qkr

    def emit_qtrans(sc, qkr):
        dests = [QT[0], QT[1], KT]
        for hh in range(3):
            pt = psC.tile([128, 128], F32, tag="psC", name=f"ptq{sc}_{hh}")
            nc.tensor.transpose(pt[:], qkr[:, hh * 128:(hh + 1) * 128],
                                idtf[:])
            if hh == 1:
                nc.vector.tensor_copy(dests[hh][:, sc * 128:(sc + 1) * 128],
                                      pt[:])
            else:
                nc.scalar.copy(dests[hh][:, sc * 128:(sc + 1) * 128], pt[:])

    # ---- Phase 3 helpers: attention, transposed orientation ----
    def emit_scores(h, t, kc):
        jlo = max(0, kc - 4 * t)
        jhi = min(3, kc + 8 - 4 * t)
        a, b = jlo * 128, (jhi + 1) * 128
        qa, qb = t * 512 + a, t * 512 + b
        sp = psA.tile([128, 512], F32, tag="psA", name=f"sp{h}_{t}_{kc}")
        nc.tensor.matmul(sp[:, a:b], KT[:, kc * 128:(kc + 1) * 128],
                         QT[h][:, qa:qb], start=True, stop=True)
        return sp, a, b

    def emit_mask_exp(h, t, kc, sp, a, b):
        j = kc - 4 * t
        if 0 <= j < 4:
            nc.vector.tensor_add(sp[:, j * 128:(j + 1) * 128],
                                 sp[:, j * 128:(j + 1) * 128], md[:])
        j2 = kc + 8 - 4 * t
        if 0 <= j2 < 4:
            nc.vector.tensor_add(sp[:, j2 * 128:(j2 + 1) * 128],
                                 sp[:, j2 * 128:(j2 + 1) * 128], me[:])
        es = espool.tile([128, 512], BF16, tag="es", name=f"es{h}_{t}_{kc}")
        nc.scalar.activation(es[:, a:b], sp[:, a:b], AF.Exp, scale=SCALE)
        return es, a, b

    # Early front-ends (score+mask+exp) for attention groups whose Q/K
    # chunks are already transposed, interleaved into the projection
    # phase: the projections keep PE busy while ACT/DVE (idle-ish there)
    # pre-compute the exp tiles, so the attention phase runs at pure
    # matmul pace.
    pre_es = {}

    def kcs_of(t):
        return list(range(max(0, 4 * t - 8), 4 * (t + 1)))

    def emit_front(h, t):
        for kc in kcs_of(t):
            sp, a, b = emit_scores(h, t, kc)
            pre_es[(h, t, kc)] = emit_mask_exp(h, t, kc, sp, a, b)

    qkr_hist = {}
    for sc in range(NSC):
        qkr_hist[sc] = emit_proj(sc)
        if sc >= 2:
            emit_qtrans(sc - 2, qkr_hist.pop(sc - 2))
    emit_qtrans(NSC - 2, qkr_hist.pop(NSC - 2))
    emit_front(0, 0)
    emit_front(1, 0)
    qkr_last = qkr_hist.pop(NSC - 1)

    def emit_wo(t, jcs=None):
        for jc in (range(16) if jcs is None else jcs):
            po = psC.tile([128, 512], F32, tag="psC", name=f"po{jc}_{t}")
            for ic in range(2):
                nc.tensor.matmul(
                    po[:], woTs[ic][:, jc * 128:(jc + 1) * 128],
                    attnT[ic][:, t * 512:(t + 1) * 512],
                    start=(ic == 0), stop=(ic == 1))
            ot = otpool.tile([128, 512], BF16, tag="ot", name=f"ot{jc}_{t}")
            if (jc + t) % 2 == 0:
                nc.vector.tensor_copy(ot[:], po[:])
            else:
                nc.scalar.copy(ot[:], po[:])
            nc.sync.dma_start(outT[jc * 128:(jc + 1) * 128,
                                   t * 512:(t + 1) * 512], ot[:])

    groups = []
    for t in range(4):
        for h in range(2):
            groups.append((h, t, list(range(max(0, 4 * t - 8), 4 * (t + 1)))))

    FRONTED = {(0, 0), (1, 0)}
    all_work = []  # flat list of (h, t, kc) still needing scores
    for h, t, kcs in groups:
        if (h, t) in FRONTED:
            continue
        for kc in kcs:
            all_work.append((h, t, kc))

    pending = {}  # (h,t,kc) -> (sp, a, b)
    LOOKAHEAD = 7
    wi = 0  # next work item to prefetch

    def prefetch(upto):
        nonlocal wi
        while wi < len(all_work) and wi < upto:
            hh, tt, kk = all_work[wi]
            pending[(hh, tt, kk)] = emit_scores(hh, tt, kk)
            wi += 1

    idx = 0
    wo_queue = []  # (t, jc) output-projection tiles awaiting emission
    for gi, (h, t, kcs) in enumerate(groups):
        op = psB.tile([128, 512], F32, tag="psB", name=f"op{h}_{t}")
        dp = psB.tile([1, 512], F32, tag="psB", name=f"dp{h}_{t}")
        for i, kc in enumerate(kcs):
            prefetch(idx + LOOKAHEAD)
            # drain queued output-projection tiles two at a time between
            # attention chunks: spreads their eviction backpressure across
            # the group instead of stalling a monolithic WO block
            if wo_queue:
                emit_wo(wo_queue[0][0], jcs=[j for _, j in wo_queue[:2]])
                del wo_queue[:2]
            if (h, t, kc) in pre_es:
                es, a, b = pre_es.pop((h, t, kc))
            else:
                sp, a, b = pending.pop((h, t, kc))
                idx += 1
                es, a, b = emit_mask_exp(h, t, kc, sp, a, b)
            first, last = (i == 0), (i == len(kcs) - 1)
            nc.tensor.matmul(dp[:, a:b], ones[:], es[:, a:b],
                             start=first, stop=last)
            nc.tensor.matmul(op[:, a:b],
                             Vb[:, kc * 128:(kc + 1) * 128],
                             es[:, a:b], start=first, stop=last)
        dn = dnpool.tile([1, 512], F32, tag="dn", name=f"dn{h}_{t}")
        nc.vector.tensor_scalar_add(dn[:], dp[:], sks[:, h:h + 1])
        nc.vector.reciprocal(dn[:], dn[:])
        db = dbpool.tile([128, 512], F32, tag="db", name=f"db{h}_{t}")
        nc.gpsimd.partition_broadcast(db[:], dn[:])
        nc.vector.tensor_mul(attnT[h][:, t * 512:(t + 1) * 512], op[:],
                             db[:])
        if gi == 1:
            # last s-chunk's transposes, deferred past the first groups so
            # its norm/rope chain never stalls the PE queue; results are
            # only needed by the t=3 groups much later.
            emit_qtrans(NSC - 1, qkr_last)
        if gi >= 2 and gi % 2 == 0:
            wo_queue.extend((t - 1, j) for j in range(16))
        if gi == len(groups) - 1:
            while wo_queue:
                emit_wo(wo_queue[0][0], jcs=[j for _, j in wo_queue[:2]])
                del wo_queue[:2]
            emit_wo(3)


_NC_CACHE = {}


def _get_nc():
    if "nc" not in _NC_CACHE:
        _NC_CACHE["nc"] = _build_kernel()
    return _NC_CACHE["nc"]


def kernel(x, cos, sin, wq, wk, wv, wo, sinks, q_norm_w, k_norm_w):
    x = np.asarray(x, np.float32).reshape(S, HID)
    xTh = np.ascontiguousarray(x.T).astype(NBF)
    cos = np.asarray(cos, np.float32)
    sin = np.asarray(sin, np.float32)
    wq = np.asarray(wq, np.float32)
    wk = np.asarray(wk, np.float32)
    wv = np.asarray(wv, np.float32)
    wo = np.asarray(wo, np.float32)
    sinks = np.asarray(sinks, np.float32)
    qw = np.asarray(q_norm_w, np.float32)
    kw = np.asarray(k_norm_w, np.float32)

    qwr = np.roll(qw, -64)
    kwr = np.roll(kw, -64)
    cs3 = np.ascontiguousarray(
        np.concatenate([cos * qw, cos * qw, cos * kw], axis=1)).astype(NBF)
    sn3 = np.ascontiguousarray(
        np.concatenate([sin * qwr, sin * qwr, sin * kwr], axis=1)).astype(NBF)
    kk = np.arange(128)[:, None]
    qq = np.arange(128)[None, :]
    maskd = np.where(kk <= qq, 0.0, NEG).astype(np.float32)
    maske = np.where(kk >= qq, 0.0, NEG).astype(np.float32)
    ident = np.eye(128, dtype=np.float32)

    in_maps = []
    for c in range(NCORES):
        kvh = c // 2
        wqkv = np.concatenate([wq[2 * c * 128:(2 * c + 2) * 128, :],
                               wk[kvh * 128:(kvh + 1) * 128, :],
                               wv[kvh * 128:(kvh + 1) * 128, :]], axis=0)
        wqkvT = np.ascontiguousarray(wqkv.T).astype(NBF)
        woT = np.ascontiguousarray(wo[:, c * 256:(c + 1) * 256].T).astype(NBF)
        in_maps.append(dict(
            xT=xTh, wqkvT=wqkvT, woT=woT, cs3=cs3, sn3=sn3,
            sinks2=np.ascontiguousarray(
                np.exp(sinks[2 * c:2 * c + 2]).reshape(1, 2)),
            maskd=maskd, maske=maske, identf=ident,
            ones1=np.ones((128, 1), NBF)))

    nc = _get_nc()
    res = run_bass_kernel_spmd(nc, in_maps, core_ids=list(range(NCORES)))
    total = res.results[0]["outT"].astype(np.float32)
    for c in range(1, NCORES):
        total = total + res.results[c]["outT"].astype(np.float32)
    return np.ascontiguousarray(total.T).reshape(1, S, HID)


# revision 73
# speedup vs baseline: 1.2797x; 1.0099x over previous
"""Sparse (sliding-window + sink) GQA attention on 8 NeuronCores.

Sharding: tensor-parallel over heads. Core c owns q-heads {2c, 2c+1} and
kv-head c//2. Each core computes its heads' attention and a partial
output projection (wo columns for its heads); host sums the 8 partials.

Data plane is bf16 (matmul inputs, DMA traffic); softmax and PSUM stay
f32. Attention runs in transposed orientation ST[k, q] so the P@V
contraction needs no on-chip transposes of the probability matrix; the
softmax denominator comes from a ones-vector matmul, and the final
normalization is folded into the PSUM->SBUF eviction of the output.
The kernel returns out^T in bf16; the host upcasts, sums and
transposes back.
"""

import numpy as np
from contextlib import ExitStack

import ml_dtypes
import concourse.bass as bass
import concourse.bacc as bacc
import concourse.mybir as mybir
import concourse.tile as tile
from concourse.bass_utils import run_bass_kernel_spmd

S = 2048
H = 16
KVH = 4
D = 128
HID = H * D
WIN = 1024
EPS = 1e-5
NCORES = 8
F32 = mybir.dt.float32
BF16 = mybir.dt.bfloat16
AF = mybir.ActivationFunctionType
SCALE = 1.0 / float(np.sqrt(D))
NEG = -1e9
NSC = S // 128  # 16 s-chunks
NEC = HID // 128  # 16 e-chunks
NBF = ml_dtypes.bfloat16


def _build_kernel():
    nc = bacc.Bacc("TRN2", target_bir_lowering=False, debug=False)

    xT = nc.dram_tensor("xT", [HID, S], BF16, kind="ExternalInput").ap()
    wqkvT = nc.dram_tensor("wqkvT", [HID, 512], BF16, kind="ExternalInput").ap()
    woT = nc.dram_tensor("woT", [256, HID], BF16, kind="ExternalInput").ap()
    cs3 = nc.dram_tensor("cs3", [S, 384], BF16, kind="ExternalInput").ap()
    sn3 = nc.dram_tensor("sn3", [S, 384], BF16, kind="ExternalInput").ap()
    sinks2 = nc.dram_tensor("sinks2", [1, 2], F32, kind="ExternalInput").ap()
    maskd = nc.dram_tensor("maskd", [128, 128], F32, kind="ExternalInput").ap()
    maske = nc.dram_tensor("maske", [128, 128], F32, kind="ExternalInput").ap()
    ones1 = nc.dram_tensor("ones1", [128, 1], BF16, kind="ExternalInput").ap()
    identf = nc.dram_tensor("identf", [128, 128], BF16, kind="ExternalInput").ap()
    outT = nc.dram_tensor("outT", [HID, S], BF16, kind="ExternalOutput").ap()

    with tile.TileContext(nc) as tc:
        with ExitStack() as ctx:
            _emit(ctx, tc, nc, xT, wqkvT, woT, cs3, sn3, sinks2,
                  maskd, maske, ones1, identf, outT)
    nc.compile()
    return nc


def _emit(ctx, tc, nc, xT, wqkvT, woT, cs3, sn3, sinks2, maskd, maske,
          ones1, identf, outT):
    # persistent tensors
    pers = ctx.enter_context(tc.tile_pool(name="pers", bufs=1))
    # streaming pools
    qkpool = ctx.enter_context(tc.tile_pool(name="qk", bufs=6))
    smpool = ctx.enter_context(tc.tile_pool(name="small", bufs=8))
    espool = ctx.enter_context(tc.tile_pool(name="es", bufs=12))
    dnpool = ctx.enter_context(tc.tile_pool(name="dn", bufs=4))
    dbpool = ctx.enter_context(tc.tile_pool(name="db", bufs=4))
    otpool = ctx.enter_context(tc.tile_pool(name="ot", bufs=8))
    # psum pools
    psA = ctx.enter_context(tc.tile_pool(name="psA", bufs=3, space="PSUM"))
    psB = ctx.enter_context(tc.tile_pool(name="psB", bufs=3, space="PSUM"))
    psC = ctx.enter_context(tc.tile_pool(name="psC", bufs=2, space="PSUM"))

    QT = [pers.tile([128, S], BF16, tag=f"QT{h}", name=f"QT{h}") for h in range(2)]
    KT = pers.tile([128, S], BF16, tag="KT")
    Vb = pers.tile([128, S], BF16, tag="Vb")
    attnT = [pers.tile([128, S], BF16, tag=f"attnT{h}", name=f"attnT{h}") for h in range(2)]
    woTs = [pers.tile([128, S], BF16, tag=f"woT{i}", name=f"woT{i}") for i in range(2)]
    md = pers.tile([128, 128], F32, tag="maskd")
    me = pers.tile([128, 128], F32, tag="maske")
    idtf = pers.tile([128, 128], BF16, tag="identf")
    ones = pers.tile([128, 1], BF16, tag="ones")
    sks = pers.tile([1, 2], F32, tag="sinks")
    epsb = pers.tile([128, 1], F32, tag="epsb")

    # x blocks: 8 blocks of 256 seq, each [128 hid-part, 16 ec, 256 seq]
    xb = [pers.tile([128, NEC * 256], BF16, tag=f"xb{b}", name=f"xb{b}")
          for b in range(8)]
    xb3 = [t[:].rearrange("p (n d) -> p n d", n=NEC) for t in xb]
    wb = pers.tile([128, NEC * 512], BF16, tag="wb")
    wb3 = wb[:].rearrange("p (n d) -> p n d", n=NEC)
    csb = pers.tile([128, NSC * 384], BF16, tag="csb")
    csb3 = csb[:].rearrange("p (n d) -> p n d", n=NSC)
    snb = pers.tile([128, NSC * 384], BF16, tag="snb")
    snb3 = snb[:].rearrange("p (n d) -> p n d", n=NSC)

    nc.vector.memset(epsb[:], EPS)
    # Dummy Sqrt as the very first activation: the table-load pass then
    # loads the sqrt_and_others set once up front (it also covers the
    # Copy/Square the projection phase uses), instead of switching
    # tables mid-projection and stalling the norm chain.
    dum = pers.tile([128, 1], F32, tag="dum")
    nc.scalar.activation(dum[:], epsb[:], AF.Sqrt)

    # ---- Phase 1: DMA loads (weights first: they gate the first chain) ----
    xT3 = xT.rearrange("(n p) s -> p n s", p=128)
    wq3 = wqkvT.rearrange("(n p) d -> p n d", p=128)
    cs3r = cs3.rearrange("(n p) d -> p n d", p=128)
    sn3r = sn3.rearrange("(n p) d -> p n d", p=128)
    nc.sync.dma_start(wb3[:, 0:1, :], wq3[:, 0:1, :])
    nc.sync.dma_start(xb3[0][:, 0:1, :], xT3[:, 0:1, 0:256])
    nc.sync.dma_start(wb3[:, 1:4, :], wq3[:, 1:4, :])
    nc.sync.dma_start(xb3[0][:, 1:8, :], xT3[:, 1:8, 0:256])
    nc.sync.dma_start(wb3[:, 4:8, :], wq3[:, 4:8, :])
    nc.sync.dma_start(xb3[0][:, 8:16, :], xT3[:, 8:16, 0:256])
    nc.sync.dma_start(wb3[:, 8:12, :], wq3[:, 8:12, :])
    nc.sync.dma_start(wb3[:, 12:16, :], wq3[:, 12:16, :])
    nc.sync.dma_start(idtf[:], identf[:])
    for b in range(1, 8):
        nc.sync.dma_start(xb3[b][:], xT3[:, :, b * 256:(b + 1) * 256])
        if b == 1:
            nc.sync.dma_start(csb3[:, 0:4, :], cs3r[:, 0:4, :])
            nc.sync.dma_start(snb3[:, 0:4, :], sn3r[:, 0:4, :])
        if b == 2:
            nc.sync.dma_start(csb3[:, 4:8, :], cs3r[:, 4:8, :])
            nc.sync.dma_start(snb3[:, 4:8, :], sn3r[:, 4:8, :])
            nc.sync.dma_start(md[:], maskd[:])
            nc.sync.dma_start(me[:], maske[:])
            nc.sync.dma_start(sks[:], sinks2[:])
            nc.sync.dma_start(ones[:], ones1[:])
        if b == 4:
            nc.sync.dma_start(csb3[:, 8:16, :], cs3r[:, 8:16, :])
            nc.sync.dma_start(snb3[:, 8:16, :], sn3r[:, 8:16, :])
    for i in range(2):
        nc.sync.dma_start(woTs[i][:], woT[i * 128:(i + 1) * 128, :])

    # ---- Phase 2: project, norm, rope, transpose ----
    def emit_proj(sc):
        b, lo = sc // 2, (sc % 2) * 128
        qkvp = psB.tile([128, 512], F32, tag="psB", name=f"qkvp{sc}")
        for ec in range(NEC):
            nc.tensor.matmul(qkvp[:], xb3[b][:, ec, lo:lo + 128],
                             wb3[:, ec, :],
                             start=(ec == 0), stop=(ec == NEC - 1))
        # PSUM evictions: ACT/DVE only (GPSIMD cannot access PSUM)
        qk = qkpool.tile([128, 384], BF16, tag="qk", name=f"qk{sc}")
        nc.scalar.copy(qk[:, 0:256], qkvp[:, 0:256])
        nc.vector.tensor_copy(qk[:, 256:384], qkvp[:, 256:384])
        nc.scalar.copy(Vb[:, sc * 128:(sc + 1) * 128], qkvp[:, 384:512])
        # rms norm (q0, q1, k)
        for hh in range(3):
            o = hh * 128
            sq_t = smpool.tile([128, 128], F32, tag="sq", name=f"sqr{sc}_{hh}")
            ss = smpool.tile([128, 1], F32, tag="ss", name=f"ss{sc}_{hh}")
            nc.scalar.activation(sq_t[:], qk[:, o:o + 128], AF.Square,
                                 accum_out=ss[:])
            rs = smpool.tile([128, 1], F32, tag="rs", name=f"rs{sc}_{hh}")
            nc.scalar.activation(rs[:], ss[:], AF.Sqrt, bias=epsb[:],
                                 scale=1.0 / D)
            iv = smpool.tile([128, 1], F32, tag="iv", name=f"iv{sc}_{hh}")
            nc.vector.reciprocal(iv[:], rs[:])
            nc.vector.tensor_scalar_mul(qk[:, o:o + 128], qk[:, o:o + 128],
                                        iv[:])
        # rope
        rot = qkpool.tile([128, 384], BF16, tag="rot", name=f"rot{sc}")
        for hh in range(3):
            o = hh * 128
            nc.vector.tensor_scalar_mul(rot[:, o:o + 64],
                                        qk[:, o + 64:o + 128], -1.0)
            nc.vector.tensor_copy(rot[:, o + 64:o + 128], qk[:, o:o + 64])
        qkr = qkpool.tile([128, 384], BF16, tag="qkr", name=f"qkr{sc}")
        nc.vector.tensor_mul(qkr[:], qk[:], csb3[:, sc, :])
        nc.vector.tensor_mul(rot[:], rot[:], snb3[:, sc, :])
        nc.vector.tensor_add(qkr[:], qkr[:], rot[:])
        return qkr

    def emit_qtrans(sc, qkr):
        dests = [QT[0], QT[1], KT]
        for hh in range(3):
            pt = psC.tile([128, 128], BF16, tag="psC", name=f"ptq{sc}_{hh}")
            nc.tensor.transpose(pt[:], qkr[:, hh * 128:(hh + 1) * 128],
                                idtf[:])
            if hh == 1:
                nc.vector.tensor_copy(dests[hh][:, sc * 128:(sc + 1) * 128],
                                      pt[:])
            else:
                nc.scalar.copy(dests[hh][:, sc * 128:(sc + 1) * 128], pt[:])

    # ---- Phase 3 helpers: attention, transposed orientation ----
    def emit_scores(h, t, kc):
        jlo = max(0, kc - 4 * t)
        jhi = min(3, kc + 8 - 4 * t)
        a, b = jlo * 128, (jhi + 1) * 128
        qa, qb = t * 512 + a, t * 512 + b
        sp = psA.tile([128, 512], F32, tag="psA", name=f"sp{h}_{t}_{kc}")
        nc.tensor.matmul(sp[:, a:b], KT[:, kc * 128:(kc + 1) * 128],
                         QT[h][:, qa:qb], start=True, stop=True)
        return sp, a, b

    def emit_mask_exp(h, t, kc, sp, a, b):
        j = kc - 4 * t
        if 0 <= j < 4:
            nc.vector.tensor_add(sp[:, j * 128:(j + 1) * 128],
                                 sp[:, j * 128:(j + 1) * 128], md[:])
        j2 = kc + 8 - 4 * t
        if 0 <= j2 < 4:
            nc.vector.tensor_add(sp[:, j2 * 128:(j2 + 1) * 128],
                                 sp[:, j2 * 128:(j2 + 1) * 128], me[:])
        es = espool.tile([128, 512], BF16, tag="es", name=f"es{h}_{t}_{kc}")
        nc.scalar.activation(es[:, a:b], sp[:, a:b], AF.Exp, scale=SCALE)
        return es, a, b

    # Early front-ends (score+mask+exp) for attention groups whose Q/K
    # chunks are already transposed, interleaved into the projection
    # phase: the projections keep PE busy while ACT/DVE (idle-ish there)
    # pre-compute the exp tiles, so the attention phase runs at pure
    # matmul pace.
    pre_es = {}

    def kcs_of(t):
        return list(range(max(0, 4 * t - 8), 4 * (t + 1)))

    def emit_front(h, t):
        for kc in kcs_of(t):
            sp, a, b = emit_scores(h, t, kc)
            pre_es[(h, t, kc)] = emit_mask_exp(h, t, kc, sp, a, b)

    qkr_hist = {}
    for sc in range(NSC):
        qkr_hist[sc] = emit_proj(sc)
        if sc >= 2:
            emit_qtrans(sc - 2, qkr_hist.pop(sc - 2))
    emit_qtrans(NSC - 2, qkr_hist.pop(NSC - 2))
    emit_front(0, 0)
    emit_front(1, 0)
    qkr_last = qkr_hist.pop(NSC - 1)

    def emit_wo(t, jcs=None):
        for jc in (range(16) if jcs is None else jcs):
            po = psC.tile([128, 512], F32, tag="psC", name=f"po{jc}_{t}")
            for ic in range(2):
                nc.tensor.matmul(
                    po[:], woTs[ic][:, jc * 128:(jc + 1) * 128],
                    attnT[ic][:, t * 512:(t + 1) * 512],
                    start=(ic == 0), stop=(ic == 1))
            ot = otpool.tile([128, 512], BF16, tag="ot", name=f"ot{jc}_{t}")
            if (jc + t) % 2 == 0:
                nc.vector.tensor_copy(ot[:], po[:])
            else:
                nc.scalar.copy(ot[:], po[:])
            nc.sync.dma_start(outT[jc * 128:(jc + 1) * 128,
                                   t * 512:(t + 1) * 512], ot[:])

    groups = []
    for t in range(4):
        for h in range(2):
            groups.append((h, t, list(range(max(0, 4 * t - 8), 4 * (t + 1)))))

    FRONTED = {(0, 0), (1, 0)}
    all_work = []  # flat list of (h, t, kc) still needing scores
    for h, t, kcs in groups:
        if (h, t) in FRONTED:
            continue
        for kc in kcs:
            all_work.append((h, t, kc))

    pending = {}  # (h,t,kc) -> (sp, a, b)
    LOOKAHEAD = 7
    wi = 0  # next work item to prefetch

    def prefetch(upto):
        nonlocal wi
        while wi < len(all_work) and wi < upto:
            hh, tt, kk = all_work[wi]
            pending[(hh, tt, kk)] = emit_scores(hh, tt, kk)
            wi += 1

    idx = 0
    wo_queue = []  # (t, jc) output-projection tiles awaiting emission
    for gi, (h, t, kcs) in enumerate(groups):
        op = psB.tile([128, 512], F32, tag="psB", name=f"op{h}_{t}")
        dp = psB.tile([1, 512], F32, tag="psB", name=f"dp{h}_{t}")
        for i, kc in enumerate(kcs):
            prefetch(idx + LOOKAHEAD)
            # drain queued output-projection tiles two at a time between
            # attention chunks: spreads their eviction backpressure across
            # the group instead of stalling a monolithic WO block
            if wo_queue:
                emit_wo(wo_queue[0][0], jcs=[j for _, j in wo_queue[:2]])
                del wo_queue[:2]
            if (h, t, kc) in pre_es:
                es, a, b = pre_es.pop((h, t, kc))
            else:
                sp, a, b = pending.pop((h, t, kc))
                idx += 1
                es, a, b = emit_mask_exp(h, t, kc, sp, a, b)
            first, last = (i == 0), (i == len(kcs) - 1)
            nc.tensor.matmul(dp[:, a:b], ones[:], es[:, a:b],
                             start=first, stop=last)
            nc.tensor.matmul(op[:, a:b],
                             Vb[:, kc * 128:(kc + 1) * 128],
                             es[:, a:b], start=first, stop=last)
        dn = dnpool.tile([1, 512], F32, tag="dn", name=f"dn{h}_{t}")
        nc.vector.tensor_scalar_add(dn[:], dp[:], sks[:, h:h + 1])
        nc.vector.reciprocal(dn[:], dn[:])
        db = dbpool.tile([128, 512], F32, tag="db", name=f"db{h}_{t}")
        nc.gpsimd.partition_broadcast(db[:], dn[:])
        nc.vector.tensor_mul(attnT[h][:, t * 512:(t + 1) * 512], op[:],
                             db[:])
        if gi == 1:
            # last s-chunk's transposes, deferred past the first groups so
            # its norm/rope chain never stalls the PE queue; results are
            # only needed by the t=3 groups much later.
            emit_qtrans(NSC - 1, qkr_last)
        if gi >= 2 and gi % 2 == 0:
            wo_queue.extend((t - 1, j) for j in range(16))
        if gi == len(groups) - 1:
            while wo_queue:
                emit_wo(wo_queue[0][0], jcs=[j for _, j in wo_queue[:2]])
                del wo_queue[:2]
            emit_wo(3)


_NC_CACHE = {}


def _get_nc():
    if "nc" not in _NC_CACHE:
        _NC_CACHE["nc"] = _build_kernel()
    return _NC_CACHE["nc"]


def kernel(x, cos, sin, wq, wk, wv, wo, sinks, q_norm_w, k_norm_w):
    x = np.asarray(x, np.float32).reshape(S, HID)
    xTh = np.ascontiguousarray(x.T).astype(NBF)
    cos = np.asarray(cos, np.float32)
    sin = np.asarray(sin, np.float32)
    wq = np.asarray(wq, np.float32)
    wk = np.asarray(wk, np.float32)
    wv = np.asarray(wv, np.float32)
    wo = np.asarray(wo, np.float32)
    sinks = np.asarray(sinks, np.float32)
    qw = np.asarray(q_norm_w, np.float32)
    kw = np.asarray(k_norm_w, np.float32)

    qwr = np.roll(qw, -64)
    kwr = np.roll(kw, -64)
    cs3 = np.ascontiguousarray(
        np.concatenate([cos * qw, cos * qw, cos * kw], axis=1)).astype(NBF)
    sn3 = np.ascontiguousarray(
        np.concatenate([sin * qwr, sin * qwr, sin * kwr], axis=1)).astype(NBF)
    kk = np.arange(128)[:, None]
    qq = np.arange(128)[None, :]
    maskd = np.where(kk <= qq, 0.0, NEG).astype(np.float32)
    maske = np.where(kk >= qq, 0.0, NEG).astype(np.float32)
    ident = np.eye(128, dtype=np.float32).astype(NBF)

    in_maps = []
    for c in range(NCORES):
        kvh = c // 2
        wqkv = np.concatenate([wq[2 * c * 128:(2 * c + 2) * 128, :],
                               wk[kvh * 128:(kvh + 1) * 128, :],
                               wv[kvh * 128:(kvh + 1) * 128, :]], axis=0)
        wqkvT = np.ascontiguousarray(wqkv.T).astype(NBF)
        woT = np.ascontiguousarray(wo[:, c * 256:(c + 1) * 256].T).astype(NBF)
        in_maps.append(dict(
            xT=xTh, wqkvT=wqkvT, woT=woT, cs3=cs3, sn3=sn3,
            sinks2=np.ascontiguousarray(
                np.exp(sinks[2 * c:2 * c + 2]).reshape(1, 2)),
            maskd=maskd, maske=maske, identf=ident,
            ones1=np.ones((128, 1), NBF)))

    nc = _get_nc()
    res = run_bass_kernel_spmd(nc, in_maps, core_ids=list(range(NCORES)))
    total = res.results[0]["outT"].astype(np.float32)
    for c in range(1, NCORES):
        total = total + res.results[c]["outT"].astype(np.float32)
    return np.ascontiguousarray(total.T).reshape(1, S, HID)


# revision 77
# speedup vs baseline: 1.2904x; 1.0084x over previous
"""Sparse (sliding-window + sink) GQA attention on 8 NeuronCores.

Sharding: tensor-parallel over heads. Core c owns q-heads {2c, 2c+1} and
kv-head c//2. Each core computes its heads' attention and a partial
output projection (wo columns for its heads); host sums the 8 partials.

Data plane is bf16 (matmul inputs, DMA traffic); softmax and PSUM stay
f32. Attention runs in transposed orientation ST[k, q] so the P@V
contraction needs no on-chip transposes of the probability matrix; the
softmax denominator comes from a ones-vector matmul, and the final
normalization is folded into the PSUM->SBUF eviction of the output.
The kernel returns out^T in bf16; the host upcasts, sums and
transposes back.
"""

import numpy as np
from contextlib import ExitStack

import ml_dtypes
import concourse.bass as bass
import concourse.bacc as bacc
import concourse.mybir as mybir
import concourse.tile as tile
from concourse.bass_utils import run_bass_kernel_spmd

S = 2048
H = 16
KVH = 4
D = 128
HID = H * D
WIN = 1024
EPS = 1e-5
NCORES = 8
F32 = mybir.dt.float32
BF16 = mybir.dt.bfloat16
AF = mybir.ActivationFunctionType
SCALE = 1.0 / float(np.sqrt(D))
NEG = -1e9
NSC = S // 128  # 16 s-chunks
NEC = HID // 128  # 16 e-chunks
NBF = ml_dtypes.bfloat16


def _build_kernel():
    nc = bacc.Bacc("TRN2", target_bir_lowering=False, debug=False)

    xT = nc.dram_tensor("xT", [HID, S], BF16, kind="ExternalInput").ap()
    wqkvT = nc.dram_tensor("wqkvT", [HID, 512], BF16, kind="ExternalInput").ap()
    woT = nc.dram_tensor("woT", [256, HID], BF16, kind="ExternalInput").ap()
    cs3 = nc.dram_tensor("cs3", [S, 384], BF16, kind="ExternalInput").ap()
    sn3 = nc.dram_tensor("sn3", [S, 384], BF16, kind="ExternalInput").ap()
    sinks2 = nc.dram_tensor("sinks2", [1, 2], F32, kind="ExternalInput").ap()
    maskd = nc.dram_tensor("maskd", [128, 128], F32, kind="ExternalInput").ap()
    maske = nc.dram_tensor("maske", [128, 128], F32, kind="ExternalInput").ap()
    ones1 = nc.dram_tensor("ones1", [128, 1], BF16, kind="ExternalInput").ap()
    identf = nc.dram_tensor("identf", [128, 128], BF16, kind="ExternalInput").ap()
    outT = nc.dram_tensor("outT", [HID, S], BF16, kind="ExternalOutput").ap()

    with tile.TileContext(nc) as tc:
        with ExitStack() as ctx:
            _emit(ctx, tc, nc, xT, wqkvT, woT, cs3, sn3, sinks2,
                  maskd, maske, ones1, identf, outT)
    nc.compile()
    return nc


def _emit(ctx, tc, nc, xT, wqkvT, woT, cs3, sn3, sinks2, maskd, maske,
          ones1, identf, outT):
    # persistent tensors
    pers = ctx.enter_context(tc.tile_pool(name="pers", bufs=1))
    # streaming pools
    qkpool = ctx.enter_context(tc.tile_pool(name="qk", bufs=6))
    smpool = ctx.enter_context(tc.tile_pool(name="small", bufs=8))
    espool = ctx.enter_context(tc.tile_pool(name="es", bufs=12))
    dnpool = ctx.enter_context(tc.tile_pool(name="dn", bufs=4))
    dbpool = ctx.enter_context(tc.tile_pool(name="db", bufs=4))
    otpool = ctx.enter_context(tc.tile_pool(name="ot", bufs=8))
    # psum pools
    psA = ctx.enter_context(tc.tile_pool(name="psA", bufs=3, space="PSUM"))
    psB = ctx.enter_context(tc.tile_pool(name="psB", bufs=3, space="PSUM"))
    psC = ctx.enter_context(tc.tile_pool(name="psC", bufs=2, space="PSUM"))

    QT = [pers.tile([128, S], BF16, tag=f"QT{h}", name=f"QT{h}") for h in range(2)]
    KT = pers.tile([128, S], BF16, tag="KT")
    Vb = pers.tile([128, S], BF16, tag="Vb")
    attnT = [pers.tile([128, S], BF16, tag=f"attnT{h}", name=f"attnT{h}") for h in range(2)]
    woTs = [pers.tile([128, S], BF16, tag=f"woT{i}", name=f"woT{i}") for i in range(2)]
    md = pers.tile([128, 128], F32, tag="maskd")
    me = pers.tile([128, 128], F32, tag="maske")
    idtf = pers.tile([128, 128], BF16, tag="identf")
    ones = pers.tile([128, 1], BF16, tag="ones")
    sks = pers.tile([1, 2], F32, tag="sinks")
    epsb = pers.tile([128, 1], F32, tag="epsb")

    # x blocks: 8 blocks of 256 seq, each [128 hid-part, 16 ec, 256 seq]
    xb = [pers.tile([128, NEC * 256], BF16, tag=f"xb{b}", name=f"xb{b}")
          for b in range(8)]
    xb3 = [t[:].rearrange("p (n d) -> p n d", n=NEC) for t in xb]
    wb = pers.tile([128, NEC * 512], BF16, tag="wb")
    wb3 = wb[:].rearrange("p (n d) -> p n d", n=NEC)
    csb = pers.tile([128, NSC * 384], BF16, tag="csb")
    csb3 = csb[:].rearrange("p (n d) -> p n d", n=NSC)
    snb = pers.tile([128, NSC * 384], BF16, tag="snb")
    snb3 = snb[:].rearrange("p (n d) -> p n d", n=NSC)

    nc.vector.memset(epsb[:], EPS)
    # PE warm-up: dependency-free matmuls on an uninitialized scratch tile
    # keep the tensor engine continuously busy through the initial DMA
    # wait, so the first real projection runs at full (ramped) clock
    # instead of the mid-pstate penalty. Results land in a PSUM tile that
    # is never read.
    warm = pers.tile([128, 512], BF16, tag="warm")
    nc.vector.memset(warm[:], 0.0)
    for w in range(8):
        wp = psA.tile([128, 512], F32, tag="psA", name=f"warm{w}")
        nc.tensor.matmul(wp[:], warm[:, 0:128], warm[:],
                         start=True, stop=True)
    # Dummy Sqrt as the very first activation: the table-load pass then
    # loads the sqrt_and_others set once up front (it also covers the
    # Copy/Square the projection phase uses), instead of switching
    # tables mid-projection and stalling the norm chain.
    dum = pers.tile([128, 1], F32, tag="dum")
    nc.scalar.activation(dum[:], epsb[:], AF.Sqrt)

    # ---- Phase 1: DMA loads (weights first: they gate the first chain) ----
    xT3 = xT.rearrange("(n p) s -> p n s", p=128)
    wq3 = wqkvT.rearrange("(n p) d -> p n d", p=128)
    cs3r = cs3.rearrange("(n p) d -> p n d", p=128)
    sn3r = sn3.rearrange("(n p) d -> p n d", p=128)
    nc.sync.dma_start(wb3[:, 0:1, :], wq3[:, 0:1, :])
    nc.sync.dma_start(xb3[0][:, 0:1, :], xT3[:, 0:1, 0:256])
    nc.sync.dma_start(wb3[:, 1:4, :], wq3[:, 1:4, :])
    nc.sync.dma_start(xb3[0][:, 1:8, :], xT3[:, 1:8, 0:256])
    nc.sync.dma_start(wb3[:, 4:8, :], wq3[:, 4:8, :])
    nc.sync.dma_start(xb3[0][:, 8:16, :], xT3[:, 8:16, 0:256])
    nc.sync.dma_start(wb3[:, 8:12, :], wq3[:, 8:12, :])
    nc.sync.dma_start(wb3[:, 12:16, :], wq3[:, 12:16, :])
    nc.sync.dma_start(idtf[:], identf[:])
    for b in range(1, 8):
        nc.sync.dma_start(xb3[b][:], xT3[:, :, b * 256:(b + 1) * 256])
        if b == 1:
            nc.sync.dma_start(csb3[:, 0:4, :], cs3r[:, 0:4, :])
            nc.sync.dma_start(snb3[:, 0:4, :], sn3r[:, 0:4, :])
        if b == 2:
            nc.sync.dma_start(csb3[:, 4:8, :], cs3r[:, 4:8, :])
            nc.sync.dma_start(snb3[:, 4:8, :], sn3r[:, 4:8, :])
            nc.sync.dma_start(md[:], maskd[:])
            nc.sync.dma_start(me[:], maske[:])
            nc.sync.dma_start(sks[:], sinks2[:])
            nc.sync.dma_start(ones[:], ones1[:])
        if b == 4:
            nc.sync.dma_start(csb3[:, 8:16, :], cs3r[:, 8:16, :])
            nc.sync.dma_start(snb3[:, 8:16, :], sn3r[:, 8:16, :])
    for i in range(2):
        nc.sync.dma_start(woTs[i][:], woT[i * 128:(i + 1) * 128, :])

    # ---- Phase 2: project, norm, rope, transpose ----
    def emit_proj(sc):
        b, lo = sc // 2, (sc % 2) * 128
        qkvp = psB.tile([128, 512], F32, tag="psB", name=f"qkvp{sc}")
        for ec in range(NEC):
            nc.tensor.matmul(qkvp[:], xb3[b][:, ec, lo:lo + 128],
                             wb3[:, ec, :],
                             start=(ec == 0), stop=(ec == NEC - 1))
        # PSUM evictions: ACT/DVE only (GPSIMD cannot access PSUM)
        qk = qkpool.tile([128, 384], BF16, tag="qk", name=f"qk{sc}")
        nc.scalar.copy(qk[:, 0:256], qkvp[:, 0:256])
        nc.vector.tensor_copy(qk[:, 256:384], qkvp[:, 256:384])
        nc.scalar.copy(Vb[:, sc * 128:(sc + 1) * 128], qkvp[:, 384:512])
        # rms norm (q0, q1, k)
        for hh in range(3):
            o = hh * 128
            sq_t = smpool.tile([128, 128], F32, tag="sq", name=f"sqr{sc}_{hh}")
            ss = smpool.tile([128, 1], F32, tag="ss", name=f"ss{sc}_{hh}")
            nc.scalar.activation(sq_t[:], qk[:, o:o + 128], AF.Square,
                                 accum_out=ss[:])
            rs = smpool.tile([128, 1], F32, tag="rs", name=f"rs{sc}_{hh}")
            nc.scalar.activation(rs[:], ss[:], AF.Sqrt, bias=epsb[:],
                                 scale=1.0 / D)
            iv = smpool.tile([128, 1], F32, tag="iv", name=f"iv{sc}_{hh}")
            nc.vector.reciprocal(iv[:], rs[:])
            nc.vector.tensor_scalar_mul(qk[:, o:o + 128], qk[:, o:o + 128],
                                        iv[:])
        # rope
        rot = qkpool.tile([128, 384], BF16, tag="rot", name=f"rot{sc}")
        for hh in range(3):
            o = hh * 128
            nc.vector.tensor_scalar_mul(rot[:, o:o + 64],
                                        qk[:, o + 64:o + 128], -1.0)
            nc.vector.tensor_copy(rot[:, o + 64:o + 128], qk[:, o:o + 64])
        qkr = qkpool.tile([128, 384], BF16, tag="qkr", name=f"qkr{sc}")
        nc.vector.tensor_mul(qkr[:], qk[:], csb3[:, sc, :])
        nc.vector.tensor_mul(rot[:], rot[:], snb3[:, sc, :])
        nc.vector.tensor_add(qkr[:], qkr[:], rot[:])
        return qkr

    def emit_qtrans(sc, qkr):
        dests = [QT[0], QT[1], KT]
        for hh in range(3):
            pt = psC.tile([128, 128], BF16, tag="psC", name=f"ptq{sc}_{hh}")
            nc.tensor.transpose(pt[:], qkr[:, hh * 128:(hh + 1) * 128],
                                idtf[:])
            if hh == 1:
                nc.vector.tensor_copy(dests[hh][:, sc * 128:(sc + 1) * 128],
                                      pt[:])
            else:
                nc.scalar.copy(dests[hh][:, sc * 128:(sc + 1) * 128], pt[:])

    # ---- Phase 3 helpers: attention, transposed orientation ----
    def emit_scores(h, t, kc):
        jlo = max(0, kc - 4 * t)
        jhi = min(3, kc + 8 - 4 * t)
        a, b = jlo * 128, (jhi + 1) * 128
        qa, qb = t * 512 + a, t * 512 + b
        sp = psA.tile([128, 512], F32, tag="psA", name=f"sp{h}_{t}_{kc}")
        nc.tensor.matmul(sp[:, a:b], KT[:, kc * 128:(kc + 1) * 128],
                         QT[h][:, qa:qb], start=True, stop=True)
        return sp, a, b

    def emit_mask_exp(h, t, kc, sp, a, b):
        j = kc - 4 * t
        if 0 <= j < 4:
            nc.vector.tensor_add(sp[:, j * 128:(j + 1) * 128],
                                 sp[:, j * 128:(j + 1) * 128], md[:])
        j2 = kc + 8 - 4 * t
        if 0 <= j2 < 4:
            nc.vector.tensor_add(sp[:, j2 * 128:(j2 + 1) * 128],
                                 sp[:, j2 * 128:(j2 + 1) * 128], me[:])
        es = espool.tile([128, 512], BF16, tag="es", name=f"es{h}_{t}_{kc}")
        nc.scalar.activation(es[:, a:b], sp[:, a:b], AF.Exp, scale=SCALE)
        return es, a, b

    # Early front-ends (score+mask+exp) for attention groups whose Q/K
    # chunks are already transposed, interleaved into the projection
    # phase: the projections keep PE busy while ACT/DVE (idle-ish there)
    # pre-compute the exp tiles, so the attention phase runs at pure
    # matmul pace.
    pre_es = {}

    def kcs_of(t):
        return list(range(max(0, 4 * t - 8), 4 * (t + 1)))

    def emit_front(h, t):
        for kc in kcs_of(t):
            sp, a, b = emit_scores(h, t, kc)
            pre_es[(h, t, kc)] = emit_mask_exp(h, t, kc, sp, a, b)

    qkr_hist = {}
    for sc in range(NSC):
        qkr_hist[sc] = emit_proj(sc)
        if sc >= 2:
            emit_qtrans(sc - 2, qkr_hist.pop(sc - 2))
    emit_qtrans(NSC - 2, qkr_hist.pop(NSC - 2))
    emit_front(0, 0)
    emit_front(1, 0)
    qkr_last = qkr_hist.pop(NSC - 1)

    def emit_wo(t, jcs=None):
        for jc in (range(16) if jcs is None else jcs):
            po = psC.tile([128, 512], F32, tag="psC", name=f"po{jc}_{t}")
            for ic in range(2):
                nc.tensor.matmul(
                    po[:], woTs[ic][:, jc * 128:(jc + 1) * 128],
                    attnT[ic][:, t * 512:(t + 1) * 512],
                    start=(ic == 0), stop=(ic == 1))
            ot = otpool.tile([128, 512], BF16, tag="ot", name=f"ot{jc}_{t}")
            if (jc + t) % 2 == 0:
                nc.vector.tensor_copy(ot[:], po[:])
            else:
                nc.scalar.copy(ot[:], po[:])
            nc.sync.dma_start(outT[jc * 128:(jc + 1) * 128,
                                   t * 512:(t + 1) * 512], ot[:])

    groups = []
    for t in range(4):
        for h in range(2):
            groups.append((h, t, list(range(max(0, 4 * t - 8), 4 * (t + 1)))))

    FRONTED = {(0, 0), (1, 0)}
    all_work = []  # flat list of (h, t, kc) still needing scores
    for h, t, kcs in groups:
        if (h, t) in FRONTED:
            continue
        for kc in kcs:
            all_work.append((h, t, kc))

    pending = {}  # (h,t,kc) -> (sp, a, b)
    LOOKAHEAD = 7
    wi = 0  # next work item to prefetch

    def prefetch(upto):
        nonlocal wi
        while wi < len(all_work) and wi < upto:
            hh, tt, kk = all_work[wi]
            pending[(hh, tt, kk)] = emit_scores(hh, tt, kk)
            wi += 1

    idx = 0
    wo_queue = []  # (t, jc) output-projection tiles awaiting emission
    for gi, (h, t, kcs) in enumerate(groups):
        op = psB.tile([128, 512], F32, tag="psB", name=f"op{h}_{t}")
        dp = psB.tile([1, 512], F32, tag="psB", name=f"dp{h}_{t}")
        for i, kc in enumerate(kcs):
            prefetch(idx + LOOKAHEAD)
            # drain queued output-projection tiles two at a time between
            # attention chunks: spreads their eviction backpressure across
            # the group instead of stalling a monolithic WO block
            if wo_queue:
                emit_wo(wo_queue[0][0], jcs=[j for _, j in wo_queue[:2]])
                del wo_queue[:2]
            if (h, t, kc) in pre_es:
                es, a, b = pre_es.pop((h, t, kc))
            else:
                sp, a, b = pending.pop((h, t, kc))
                idx += 1
                es, a, b = emit_mask_exp(h, t, kc, sp, a, b)
            first, last = (i == 0), (i == len(kcs) - 1)
            nc.tensor.matmul(dp[:, a:b], ones[:], es[:, a:b],
                             start=first, stop=last)
            nc.tensor.matmul(op[:, a:b],
                             Vb[:, kc * 128:(kc + 1) * 128],
                             es[:, a:b], start=first, stop=last)
        dn = dnpool.tile([1, 512], F32, tag="dn", name=f"dn{h}_{t}")
        nc.vector.tensor_scalar_add(dn[:], dp[:], sks[:, h:h + 1])
        nc.vector.reciprocal(dn[:], dn[:])
        db = dbpool.tile([128, 512], F32, tag="db", name=f"db{h}_{t}")
        nc.gpsimd.partition_broadcast(db[:], dn[:])
        nc.vector.tensor_mul(attnT[h][:, t * 512:(t + 1) * 512], op[:],
                             db[:])
        if gi == 1:
            # last s-chunk's transposes, deferred past the first groups so
            # its norm/rope chain never stalls the PE queue; results are
            # only needed by the t=3 groups much later.
            emit_qtrans(NSC - 1, qkr_last)
        if gi >= 2 and gi % 2 == 0:
            wo_queue.extend((t - 1, j) for j in range(16))
        if gi == len(groups) - 1:
            while wo_queue:
                emit_wo(wo_queue[0][0], jcs=[j for _, j in wo_queue[:2]])
                del wo_queue[:2]
            emit_wo(3)


_NC_CACHE = {}


def _get_nc():
    if "nc" not in _NC_CACHE:
        _NC_CACHE["nc"] = _build_kernel()
    return _NC_CACHE["nc"]


def kernel(x, cos, sin, wq, wk, wv, wo, sinks, q_norm_w, k_norm_w):
    x = np.asarray(x, np.float32).reshape(S, HID)
    xTh = np.ascontiguousarray(x.T).astype(NBF)
    cos = np.asarray(cos, np.float32)
    sin = np.asarray(sin, np.float32)
    wq = np.asarray(wq, np.float32)
    wk = np.asarray(wk, np.float32)
    wv = np.asarray(wv, np.float32)
    wo = np.asarray(wo, np.float32)
    sinks = np.asarray(sinks, np.float32)
    qw = np.asarray(q_norm_w, np.float32)
    kw = np.asarray(k_norm_w, np.float32)

    qwr = np.roll(qw, -64)
    kwr = np.roll(kw, -64)
    cs3 = np.ascontiguousarray(
        np.concatenate([cos * qw, cos * qw, cos * kw], axis=1)).astype(NBF)
    sn3 = np.ascontiguousarray(
        np.concatenate([sin * qwr, sin * qwr, sin * kwr], axis=1)).astype(NBF)
    kk = np.arange(128)[:, None]
    qq = np.arange(128)[None, :]
    maskd = np.where(kk <= qq, 0.0, NEG).astype(np.float32)
    maske = np.where(kk >= qq, 0.0, NEG).astype(np.float32)
    ident = np.eye(128, dtype=np.float32).astype(NBF)

    in_maps = []
    for c in range(NCORES):
        kvh = c // 2
        wqkv = np.concatenate([wq[2 * c * 128:(2 * c + 2) * 128, :],
                               wk[kvh * 128:(kvh + 1) * 128, :],
                               wv[kvh * 128:(kvh + 1) * 128, :]], axis=0)
        wqkvT = np.ascontiguousarray(wqkv.T).astype(NBF)
        woT = np.ascontiguousarray(wo[:, c * 256:(c + 1) * 256].T).astype(NBF)
        in_maps.append(dict(
            xT=xTh, wqkvT=wqkvT, woT=woT, cs3=cs3, sn3=sn3,
            sinks2=np.ascontiguousarray(
                np.exp(sinks[2 * c:2 * c + 2]).reshape(1, 2)),
            maskd=maskd, maske=maske, identf=ident,
            ones1=np.ones((128, 1), NBF)))

    nc = _get_nc()
    res = run_bass_kernel_spmd(nc, in_maps, core_ids=list(range(NCORES)))
    total = res.results[0]["outT"].astype(np.float32)
    for c in range(1, NCORES):
        total = total + res.results[c]["outT"].astype(np.float32)
    return np.ascontiguousarray(total.T).reshape(1, S, HID)
